# revision 21
# baseline (speedup 1.0000x reference)
"""Causal multi-head attention Trainium2 kernel (8 NeuronCores).

Problem: B=4, L=2048, D=1024, 16 heads x (dh=64, dv=64), causal mask.
Sharding: data-parallel over batch (4) x tensor-parallel over heads (2 groups
of 8). Core c handles batch c//2, head-group c%2. Each core computes its
partial output projection (ctx_g @ Wo_g); the host sums the two head-group
partials per batch and adds the bias.

v2-v4 (~322us): software-pipelined flash-style attention; S^T tiles = K@Q^T;
exp on ACT (scale=1/8 folded in); causal diagonal via tril tensor_mul +
width-restricted matmuls; ones column of V_aug gives the softmax denominator
in PSUM row 64; bf16 datapath; projections and output projection dribbled
into the attention stream.

v5 (~309us): x arrives pre-transposed from the host; the two heads' S^T
matmuls (contraction 64: head0 on PE rows 0-63/tile T0, head1 rows 64-127/
T8) issue adjacently so the hardware overlaps them on the two 64x128 PE
tiles.

v6: (a) all DRAM operands pre-shuffled on the host into partition-contiguous
layouts (x chunk-major [nch,128,8,512], weights [128,t,o]) so the input DMAs
run at full HBM bandwidth - v5 lost ~15us waiting on 1KB-segment descriptor
patterns at startup. (b) hp3 processes q-chunks descending: the large
l-tiles' output projections dribble into the remaining attention, and the
final drain is only lt 0-3, split into an early v=0..2 partial (pA, dribbled)
plus a 1-matmul finish - the v5 drain ran ~23 matmuls after a >3us PE gap
that re-throttled the HAM clock gate to 1.2 GHz. (c) output stored as bf16
(halves the store bytes; host upcasts and sums the TP partials in f32).
(d) Wq/Wk loads ride the gpsimd queue - their engine-blocking DIRECT2D
triggers were stalling the exp stream on the ACT queue. (e) S quartet issue
order alternates heads (h0r0,h1r0,h0r1,h1r1) for tile overlap.
"""

import numpy as np
from contextlib import ExitStack

import concourse.bass as bass
import concourse.tile as tile
from concourse import bacc, mybir

F32 = mybir.dt.float32
BF16 = mybir.dt.bfloat16
AF = mybir.ActivationFunctionType

B, L, D = 4, 2048, 1024
N_HEAD, DH, DV = 16, 64, 64
N_CORES = 8
HPC = N_HEAD // 2          # heads per core (8)
OC = HPC * DH              # per-core projection width (512)
NHP = HPC // 2             # head-pairs per core (4)


class ProjEmitter:
    """Q^T/K^T projection for one head-pair, emitted in per-(proj,chunk)
    units so the matmuls interleave with attention of the previous pair."""

    def __init__(self, nc, hp, pools, xt, wq, wk, nch, start=0):
        self.nc = nc
        self.xt = xt
        self.start = start
        qkp, wp, self.psP = pools
        self.wq_sb = wp.tile([128, 8, 128], BF16, tag="wq")
        self.wk_sb = wp.tile([128, 8, 128], BF16, tag="wk")
        # gpsimd queue: keeps the engine-blocking DIRECT2D weight triggers
        # off the ACT queue (exps) and the sync queue (normalize bcasts)
        nc.gpsimd.dma_start(out=self.wq_sb, in_=wq[hp])
        nc.gpsimd.dma_start(out=self.wk_sb, in_=wk[hp])
        self.qt = qkp.tile([128, nch * 512], BF16, tag="qt")
        self.kt = qkp.tile([128, nch * 512], BF16, tag="kt")
        self.units = [(w, d, cp) for w, d in ((self.wq_sb, self.qt),
                                              (self.wk_sb, self.kt))
                      for cp in range(nch // 2)]
        self.i = 0

    def step(self):
        if self.i >= len(self.units):
            return False
        w_sb, dst, cp = self.units[self.i]
        self.i += 1
        nc = self.nc
        # one unit = two q-chunks into one 2-bank PSUM tile, drained by a
        # single wide DVE copy (the ISA caps moving width at 512)
        pp = self.psP.tile([128, 1024], F32, tag="ppq", bufs=1)
        for half in range(2):
            for d in range(8):
                nc.tensor.matmul(pp[:, half * 512:(half + 1) * 512],
                                 w_sb[:, d, :],
                                 self.xt[:, 2 * cp + half, d, :],
                                 start=(d == 0), stop=(d == 7))
        nc.vector.tensor_copy(dst[:, cp * 1024:(cp + 1) * 1024], pp)
        return True

    def drain(self):
        while self.step():
            pass


class OutEmitter:
    """Output projection, dribbled into hp3's attention (hp3 runs its
    q-chunks descending, so l-tiles 12..4 project early); the last chunk's
    tiles 0-3 are split into an early v=0..2 partial plus a single-matmul
    finish so almost nothing runs after the final normalize."""

    def __init__(self, nc, pools, ct, wo_sb, out, drain_lts):
        self.nc = nc
        self.phco, self.psP = pools
        self.ct, self.wo_sb, self.out = ct, wo_sb, out
        self.drain_lts = drain_lts
        self.start = 0
        self.queue = []
        self.pa = {}
        self.dq = 0

    def add_lt(self, lt):
        self.queue.append(("full", lt))

    def add_partial(self, lt):
        self.queue.append(("pA", lt))

    def add_finish(self, lt):
        self.queue.append(("fin", lt))

    def step(self):
        if not self.queue:
            return False
        kind, lt = self.queue.pop(0)
        nc = self.nc
        # one unit covers a whole l-tile: matmuls per 512-half into a
        # 2-bank PSUM tile, drained by one wide DVE copy
        pp = self.psP.tile([128, 1024], F32, tag="ppq", bufs=1, name="ppo")
        if kind == "pA":
            for n in range(2):
                for v in range(3):
                    nc.tensor.matmul(
                        pp[:, n * 512:(n + 1) * 512],
                        self.ct[:, v, lt * 128:(lt + 1) * 128],
                        self.wo_sb[:, v, n * 512:(n + 1) * 512],
                        start=(v == 0), stop=(v == 2))
            pa = self.phco.tile([128, D], F32, tag="pA", bufs=4,
                                name=f"pA{lt}")
            nc.vector.tensor_copy(pa, pp)
            self.pa[lt] = pa
            return True
        ost = self.phco.tile([128, D], BF16, tag="ost", name=f"ost{lt}")
        if kind == "fin":
            for n in range(2):
                nc.tensor.matmul(pp[:, n * 512:(n + 1) * 512],
                                 self.ct[:, 3, lt * 128:(lt + 1) * 128],
                                 self.wo_sb[:, 3, n * 512:(n + 1) * 512],
                                 start=True, stop=True)
            nc.vector.tensor_add(ost, pp, self.pa.pop(lt))
            # final-tail halves: store immediately, alternating the sync
            # and ACT hw DGE queues (idle by now) so the stores overlap
            for n in range(2):
                eng = nc.sync if self.dq % 2 == 0 else nc.scalar
                self.dq += 1
                eng.dma_start(
                    out=self.out[lt * 128:(lt + 1) * 128,
                                 n * 512:(n + 1) * 512],
                    in_=ost[:, n * 512:(n + 1) * 512])
            return True
        for n in range(2):
            for v in range(4):
                nc.tensor.matmul(pp[:, n * 512:(n + 1) * 512],
                                 self.ct[:, v, lt * 128:(lt + 1) * 128],
                                 self.wo_sb[:, v, n * 512:(n + 1) * 512],
                                 start=(v == 0), stop=(v == 3))
        nc.vector.tensor_copy(ost, pp)
        # alternate the (idle) gpsimd and sync queues so the ~4MiB of
        # output stores drain in parallel instead of piling up
        eng = nc.gpsimd if lt % 2 == 0 else nc.sync
        eng.dma_start(out=self.out[lt * 128:(lt + 1) * 128, :], in_=ost)
        return True

    def drain(self):
        while self.step():
            pass


def build_nc(l=L):
    assert l % 512 == 0
    nch = l // 512           # q-chunks
    nlt = l // 128           # l-tiles
    nc = bacc.Bacc("TRN2", target_bir_lowering=False, debug=False,
                   num_devices=N_CORES)

    # all operands pre-shuffled on the host into partition-contiguous
    # DMA layouts (see make_in_maps)
    x = nc.dram_tensor("x", [nch, 128, 8, 512], BF16,
                       kind="ExternalInput").ap()
    wq = nc.dram_tensor("wq", [NHP, 128, 8, 128], BF16,
                        kind="ExternalInput").ap()
    wk = nc.dram_tensor("wk", [NHP, 128, 8, 128], BF16,
                        kind="ExternalInput").ap()
    wv = nc.dram_tensor("wv", [128, 8, OC], BF16, kind="ExternalInput").ap()
    wo = nc.dram_tensor("wo", [128, 4, D], BF16, kind="ExternalInput").ap()
    out = nc.dram_tensor("out", [l, D], BF16, kind="ExternalOutput").ap()

    with tile.TileContext(nc) as tc, ExitStack() as ctx:
        top = ctx.enter_context(tc.tile_pool(name="top", bufs=1))
        xtp = ctx.enter_context(tc.tile_pool(name="xtp", bufs=1))
        qkp = ctx.enter_context(tc.tile_pool(name="qkp", bufs=2))
        wp = ctx.enter_context(tc.tile_pool(name="wp", bufs=2))
        phco = ctx.enter_context(tc.tile_pool(name="phco", bufs=4))

        # V: [128(l), ltile, head, 65] - col 64 is ones (softmax denominator)
        vt = top.tile([128, nlt, HPC, DH + 1], BF16)
        ct = top.tile([128, NHP, l], BF16)        # normalized ctx^T
        tril = top.tile([128, 128], BF16)
        ones = top.tile([128, 1], F32)
        warm = top.tile([128, 1], BF16)
        onesw = top.tile([1, DV], BF16)           # rank-1 bcast weights
        xt = xtp.tile([128, nch, 8, 512], BF16)   # x^T, chunk-major

        wones = top.tile([128, 1], BF16)
        wbuf = top.tile([128, 256], BF16)

        nc.vector.memset(ones, 1.0)
        nc.vector.memset(onesw, 1.0)
        nc.vector.memset(wones, 1.0)
        nc.vector.memset(wbuf, 0.0)
        # warm-up exp: loads the ACT function table during the DMA-bound
        # startup instead of stalling the first attention group
        nc.scalar.activation(warm, ones, AF.Exp, scale=0.125)
        nc.vector.tensor_copy(
            vt[:, :, :, DV:DV + 1].rearrange("p t h c -> p (t h) c"),
            ones.broadcast_to((128, nlt * HPC, 1)))
        # causal keep-mask for S^T diag blocks: tril[k, q] = 1.0 iff q >= k
        nc.gpsimd.memset(tril, 0.0)
        nc.gpsimd.affine_select(
            out=tril, in_=tril, compare_op=mybir.AluOpType.is_gt,
            fill=1.0, base=0, pattern=[[-1, 128]], channel_multiplier=1)

        # ---------------- Prologue: xT DMA + V + QK(hp=0) -----------------
        with tc.tile_pool(name="wvp", bufs=1) as wvp, \
             tc.tile_pool(name="psPro", bufs=3, space="PSUM") as psPro:
            # DMA staging: the device DMA pipe is shared (8 cores pull
            # ~50MiB of inputs concurrently at kernel start, ~150-250 GB/s
            # effective per core), so ARRIVAL ORDER is everything. Transfers
            # on one queue run FIFO - putting all x chunks on sync in need
            # order stages them automatically; wv/wo ride scalar, Wq/Wk
            # gpsimd, so at most ~3 transfers share the pipe at once.
            wv_sb = wvp.tile([128, 8, OC], BF16)
            nc.scalar.dma_start(out=wv_sb[:, 0:4, :], in_=wv[:, 0:4, :])
            nc.sync.dma_start(out=xt[:, 0, :, 0:256], in_=x[0, :, :, 0:256])
            nc.scalar.dma_start(out=wv_sb[:, 4:8, :], in_=wv[:, 4:8, :])
            nc.sync.dma_start(out=xt[:, 0, :, 256:512],
                              in_=x[0, :, :, 256:512])
            for c in range(1, nch):
                nc.sync.dma_start(out=xt[:, c], in_=x[c])
            # ~24 rank-1 warm-up matmuls fill the DMA wait with PE activity
            # so the HAM clock gate reaches 8/8 before the first real matmul
            for w in range(24):
                pw = psPro.tile([1, 256], F32, tag="pwarm", bufs=2)
                nc.tensor.matmul(pw, wones, wbuf, start=True, stop=True)
            for c in range(nch):
                if c == 1:
                    # hp0's Wq/Wk ride the (idle) gpsimd queue
                    em = ProjEmitter(nc, 0, (qkp, wp, psPro), xt, wq, wk, nch)
                # V for this l-chunk
                for m in range(4):
                    pp = psPro.tile([128, 512], F32, tag="pp")
                    for d in range(8):
                        nc.tensor.matmul(
                            pp, xt[:, c, d, m * 128:(m + 1) * 128],
                            wv_sb[:, d, :], start=(d == 0), stop=(d == 7))
                    nc.vector.tensor_copy(
                        vt[:, c * 4 + m, :, 0:DV],
                        pp.rearrange("p (h v) -> p h v", h=HPC))
            em.drain()

        # Prefetch Wo now: the scalar DMA queue drains during early hp0
        # attention, long before the output projection needs it.
        phc = ctx.enter_context(tc.tile_pool(name="phc", bufs=1))
        wo_sb = phc.tile([128, 4, D], BF16)
        nc.scalar.dma_start(out=wo_sb, in_=wo)

        # ---------------- Main: attention + next-pair projections ---------
        with tc.tile_pool(name="phb", bufs=2) as phb, \
             tc.tile_pool(name="psS", bufs=2, space="PSUM") as psS, \
             tc.tile_pool(name="psPd", bufs=2, space="PSUM") as psPd, \
             tc.tile_pool(name="psC", bufs=2, space="PSUM") as psC:
            n_groups_hp = 2 * nch * (nch + 1)
            for hp in range(NHP):
                qt, kt = em.qt, em.kt
                if hp + 1 < NHP:
                    # hp0 runs j descending, so its shallow (bubble-prone)
                    # chunks come last: dribble the projection units there.
                    # Ascending hps dribble early (after the Wq/Wk DMA
                    # lands) into their shallow first chunks.
                    em = ProjEmitter(nc, hp + 1, (qkp, wp, psPd), xt, wq, wk,
                                     nch, start=24 if hp == 0 else 4)
                    cadence = 2 if hp == 0 else 1
                else:
                    em = OutEmitter(nc, (phco, psPd), ct, wo_sb, out,
                                    drain_lts=range(0, 4))
                    # lt 0-3's head-pair 0-2 ctx has been ready since hp2:
                    # dribble their v=0..2 partials during hp3's first chunk
                    for lt in range(4):
                        em.add_partial(lt)
                    cadence = 1
                gcount = 0

                def pv_step(g, j, pctx, pexp, po, H):
                    # masks + PV for group g (one group after its exp)
                    for r2 in range(2):
                        kt_i = 2 * g + r2
                        r = kt_i - 4 * j
                        c0 = 0
                        if r >= 0:      # diagonal k-tile
                            c0 = r * 128
                            nc.vector.tensor_mul(
                                pexp[:, r2, c0:c0 + 128],
                                pexp[:, r2, c0:c0 + 128], tril)
                        nc.tensor.matmul(
                            pctx[:, c0:512],
                            vt[:, kt_i, H, :],
                            pexp[:, r2, c0:512],
                            start=(kt_i == 0), stop=(kt_i == 4 * j + 3))

                # hp0 and hp3 run j descending: hp0 so j=3's 32 S matmuls
                # hide the cold-ACT exp latency at the prologue boundary,
                # hp3 so the big l-tiles' out-projection dribbles early and
                # only lt 0-3 remain for the drain.
                desc = hp == 0 or hp == NHP - 1
                jorder = range(nch - 1, -1, -1) if desc else range(nch)
                last_j = 0 if desc else nch - 1
                for j in jorder:
                    if hp == NHP - 1:
                        cadence = 2
                    n_g = 2 * (j + 1)
                    pctxs = {}
                    prevs = {0: None, 1: None}
                    for h in range(2):
                        pctxs[h] = psC.tile([DV + 1, 512], F32,
                                            tag="pctx", name=f"pctx{h}")
                    for g in range(n_g + 1):
                        # S^T for both heads interleaved r2-major: head0's
                        # matmuls run on PE rows 0-63 (tile T0), head1's on
                        # rows 64-127 (T8); with no full-array matmul
                        # between them the two 64x128 tiles overlap.
                        pscs = {}
                        if g < n_g:
                            for h in range(2):
                                pscs[h] = psS.tile([128, 2, 512], F32,
                                                   tag="psc", name=f"psc{h}")
                            for r2 in range(2):
                                kt_i = 2 * g + r2
                                c0 = max(0, kt_i - 4 * j) * 128
                                for h in range(2):
                                    po = 64 * h
                                    nc.tensor.matmul(
                                        pscs[h][:, r2, c0:512],
                                        kt[po:po + DH,
                                           kt_i * 128:(kt_i + 1) * 128],
                                        qt[po:po + DH,
                                           j * 512 + c0:(j + 1) * 512],
                                        start=True, stop=True)
                            for h in range(2):
                                pexp = phb.tile([128, 2, 512], BF16,
                                                tag="pexp", bufs=10,
                                                name=f"pexp{h}")
                                cg = max(0, 2 * g - 4 * j) * 128
                                nc.scalar.activation(pexp[:, :, cg:512],
                                                     pscs[h][:, :, cg:512],
                                                     AF.Exp, scale=0.125)
                                pscs[h] = pexp
                        for h in range(2):
                            po = 64 * h
                            H = 2 * hp + h
                            if prevs[h] is not None:
                                pv_step(prevs[h][0], j, pctxs[h],
                                        prevs[h][1], po, H)
                                gcount += 1
                                if (em and cadence and gcount > em.start
                                        and gcount % cadence == 0):
                                    em.step()
                            prevs[h] = (g, pscs[h]) if g < n_g else None
                    if hp == NHP - 1 and j == last_j:
                        # flush remaining independent out-proj units BEFORE
                        # the final normalize: the normalize-dependent rank-1
                        # broadcasts otherwise block them at the head of the
                        # strict-FIFO PE queue (and the idle re-throttles the
                        # HAM clock gate, running the whole drain at 1.2GHz)
                        em.drain()
                        # final chunk, both heads' chains interleaved:
                        # broadcast via rank-1 PE matmuls (~0.2us) instead
                        # of the ~1us gpsimd path; recip reads PSUM directly
                        # and emits bf16 so the chain is 3 DVE ops deep
                        ctus, invbs, bcps = {}, {}, {}
                        rss, invs = {}, {}
                        for h in range(2):
                            # reciprocal_approx is a bit-trick op: stage the
                            # PSUM denominator row to SBUF (IEEE f32) first
                            rss[h] = phb.tile([1, 512], F32, tag="rs",
                                              name=f"rs{h}")
                            nc.vector.tensor_copy(rss[h],
                                                  pctxs[h][DV:DV + 1, :])
                        for h in range(2):
                            ctus[h] = phb.tile([64, 512], BF16, tag="ctu",
                                               name=f"ctu{h}")
                            nc.vector.tensor_copy(ctus[h], pctxs[h][0:DV, :])
                        for h in range(2):
                            invs[h] = phb.tile([1, 512], F32, tag="inv",
                                               name=f"inv{h}")
                            nc.vector.reciprocal_approx_fast(
                                out=invs[h], in_=rss[h])
                        for h in range(2):
                            invbs[h] = phb.tile([1, 512], BF16, tag="invb",
                                                name=f"invb{h}")
                            nc.vector.tensor_copy(invbs[h], invs[h])
                        for h in range(2):
                            po = 64 * h
                            bcp = psPd.tile([128, 1024], F32, tag="ppq",
                                            bufs=1, name=f"bcp{h}")
                            nc.tensor.matmul(bcp[0:DV, 0:512], onesw,
                                             invbs[h], start=True, stop=True)
                            nc.vector.tensor_mul(
                                ct[po:po + DV, hp, j * 512:(j + 1) * 512],
                                ctus[h], bcp[0:DV, 0:512])
                    else:
                        for h in range(2):
                            po = 64 * h
                            # free the pctx PSUM bank after one DVE copy +
                            # a PSUM-direct recip (the next j's first PV
                            # waits on it); broadcast/scale dangle off the
                            # critical path.
                            rs = phb.tile([1, 512], F32, tag="rs",
                                          name="rs")
                            nc.vector.tensor_copy(rs,
                                                  pctxs[h][DV:DV + 1, :])
                            ctu = phb.tile([64, 512], BF16, tag="ctu",
                                           name=f"ctu{h}")
                            nc.vector.tensor_copy(ctu, pctxs[h][0:DV, :])
                            inv = phb.tile([1, 512], F32, tag="inv",
                                           name="inv")
                            nc.vector.reciprocal_approx_fast(
                                out=inv, in_=rs)
                            bc = phb.tile([64, 512], F32, tag="bc",
                                          name="bc")
                            nc.gpsimd.partition_broadcast(out_ap=bc,
                                                          in_ap=inv)
                            nc.vector.tensor_mul(
                                ct[po:po + DV, hp, j * 512:(j + 1) * 512],
                                ctu, bc)
                    if hp == NHP - 1:
                        if j == last_j:
                            em.add_finish(0)
                            em.add_finish(1)
                            em.add_finish(2)
                            em.add_finish(3)
                        else:
                            for lt in range(4 * j, 4 * j + 4):
                                em.add_lt(lt)
                em.drain()

    nc.compile()
    return nc


def _bf16(a):
    import ml_dtypes
    return np.ascontiguousarray(a).astype(ml_dtypes.bfloat16)


def make_in_maps(x, Wq, Wk, Wv, Wo):
    nch = L // 512
    in_maps = []
    for c in range(N_CORES):
        b, g = c // 2, c % 2
        # x^T chunk-major: [c, p, t, lc] = x[b][c*512+lc, t*128+p]
        xdev = x[b].reshape(nch, 512, 8, 128).transpose(0, 3, 2, 1)
        wqg = Wq[:, g * OC:(g + 1) * OC]
        wkg = Wk[:, g * OC:(g + 1) * OC]
        # per-head-pair [hp, p, t, o] = W[t*128+p, hp*128+o]
        wqd = wqg.reshape(8, 128, NHP, 128).transpose(2, 1, 0, 3)
        wkd = wkg.reshape(8, 128, NHP, 128).transpose(2, 1, 0, 3)
        # [p, t, o] = W[t*128+p, o]
        wvd = Wv[:, g * OC:(g + 1) * OC].reshape(8, 128, OC).transpose(1, 0, 2)
        # [p, v, o] = Wo[v*128+p, o]
        wod = Wo[g * OC:(g + 1) * OC, :].reshape(4, 128, D).transpose(1, 0, 2)
        in_maps.append({
            "x": _bf16(xdev),
            "wq": _bf16(wqd),
            "wk": _bf16(wkd),
            "wv": _bf16(wvd),
            "wo": _bf16(wod),
        })
    return in_maps


_NC_CACHE = {}


def _get_nc():
    if "nc" not in _NC_CACHE:
        _NC_CACHE["nc"] = build_nc()
    return _NC_CACHE["nc"]


def _numpy_fallback(x, Wq, Wk, Wv, Wo, bo, mask):
    Bsz, Lq, _ = x.shape
    Q = (x @ Wq).reshape(Bsz, Lq, N_HEAD, DH).transpose(0, 2, 1, 3)
    K = (x @ Wk).reshape(Bsz, Lq, N_HEAD, DH).transpose(0, 2, 1, 3)
    V = (x @ Wv).reshape(Bsz, Lq, N_HEAD, DV).transpose(0, 2, 1, 3)
    s = np.einsum("bhqd,bhkd->bhqk", Q, K) / np.sqrt(np.float32(DH))
    s = np.where(mask, s, -np.inf)
    s = s - s.max(axis=-1, keepdims=True)
    p = np.exp(s)
    p /= p.sum(axis=-1, keepdims=True)
    ctxv = np.einsum("bhqk,bhkv->bhqv", p, V)
    ctxv = ctxv.transpose(0, 2, 1, 3).reshape(Bsz, Lq, N_HEAD * DV)
    return (ctxv @ Wo + bo).astype(np.float32)


def run_on_hw(in_maps, trace=False):
    from concourse.bass_utils import run_bass_kernel_spmd
    nc = _get_nc()
    return run_bass_kernel_spmd(nc, in_maps, list(range(N_CORES)), trace=trace)


def kernel(x, Wq, Wk, Wv, Wo, bo, mask, _trace=False, _results=None):
    x = np.asarray(x, dtype=np.float32)
    Wq = np.asarray(Wq, dtype=np.float32)
    Wk = np.asarray(Wk, dtype=np.float32)
    Wv = np.asarray(Wv, dtype=np.float32)
    Wo = np.asarray(Wo, dtype=np.float32)
    bo = np.asarray(bo, dtype=np.float32)
    mask_np = np.asarray(mask).reshape(mask.shape[-2], mask.shape[-1])

    causal = bool(np.array_equal(
        mask_np, np.tril(np.ones((L, L), dtype=bool))))
    if not causal or x.shape != (B, L, D):
        return _numpy_fallback(np.asarray(x), Wq, Wk, Wv, Wo, bo,
                               np.asarray(mask))

    res = run_on_hw(make_in_maps(x, Wq, Wk, Wv, Wo), trace=_trace)
    if _results is not None:
        _results.append(res)
    out = np.empty((B, L, D), dtype=np.float32)
    for b in range(B):
        out[b] = (np.asarray(res.results[2 * b]["out"], dtype=np.float32)
                  + np.asarray(res.results[2 * b + 1]["out"], dtype=np.float32)
                  + bo)
    return out


# revision 24
# speedup vs baseline: 1.0541x; 1.0541x over previous
"""Causal multi-head attention Trainium2 kernel (8 NeuronCores).

Problem: B=4, L=2048, D=1024, 16 heads x (dh=64, dv=64), causal mask.
Sharding: data-parallel over batch (4) x tensor-parallel over heads (2 groups
of 8). Core c handles batch c//2, head-group c%2. Each core computes its
partial output projection (ctx_g @ Wo_g); the host sums the two head-group
partials per batch and adds the bias.

v2-v4 (~322us): software-pipelined flash-style attention; S^T tiles = K@Q^T;
exp on ACT (scale=1/8 folded in); causal diagonal via tril tensor_mul +
width-restricted matmuls; ones column of V_aug gives the softmax denominator
in PSUM row 64; bf16 datapath; projections and output projection dribbled
into the attention stream.

v5 (~309us): x arrives pre-transposed from the host; the two heads' S^T
matmuls (contraction 64: head0 on PE rows 0-63/tile T0, head1 rows 64-127/
T8) issue adjacently so the hardware overlaps them on the two 64x128 PE
tiles.

v6: (a) all DRAM operands pre-shuffled on the host into partition-contiguous
layouts (x chunk-major [nch,128,8,512], weights [128,t,o]) so the input DMAs
run at full HBM bandwidth - v5 lost ~15us waiting on 1KB-segment descriptor
patterns at startup. (b) hp3 processes q-chunks descending: the large
l-tiles' output projections dribble into the remaining attention, and the
final drain is only lt 0-3, split into an early v=0..2 partial (pA, dribbled)
plus a 1-matmul finish - the v5 drain ran ~23 matmuls after a >3us PE gap
that re-throttled the HAM clock gate to 1.2 GHz. (c) output stored as bf16
(halves the store bytes; host upcasts and sums the TP partials in f32).
(d) Wq/Wk loads ride the gpsimd queue - their engine-blocking DIRECT2D
triggers were stalling the exp stream on the ACT queue. (e) S quartet issue
order alternates heads (h0r0,h1r0,h0r1,h1r1) for tile overlap.
"""

import numpy as np
from contextlib import ExitStack

import concourse.bass as bass
import concourse.tile as tile
from concourse import bacc, mybir

F32 = mybir.dt.float32
BF16 = mybir.dt.bfloat16
AF = mybir.ActivationFunctionType

B, L, D = 4, 2048, 1024
N_HEAD, DH, DV = 16, 64, 64
N_CORES = 8
HPC = N_HEAD // 2          # heads per core (8)
OC = HPC * DH              # per-core projection width (512)
NHP = HPC // 2             # head-pairs per core (4)


class ProjEmitter:
    """Q^T/K^T projection for one head-pair, emitted in per-(proj,chunk)
    units so the matmuls interleave with attention of the previous pair."""

    def __init__(self, nc, hp, pools, xt, wq, wk, nch, start=0):
        self.nc = nc
        self.xt = xt
        self.start = start
        qkp, wp, self.psP = pools
        self.wq_sb = wp.tile([128, 8, 128], BF16, tag="wq")
        self.wk_sb = wp.tile([128, 8, 128], BF16, tag="wk")
        # gpsimd queue: keeps the engine-blocking DIRECT2D weight triggers
        # off the ACT queue (exps) and the sync queue (normalize bcasts)
        nc.gpsimd.dma_start(out=self.wq_sb, in_=wq[hp])
        nc.gpsimd.dma_start(out=self.wk_sb, in_=wk[hp])
        self.qt = qkp.tile([128, nch * 512], BF16, tag="qt")
        self.kt = qkp.tile([128, nch * 512], BF16, tag="kt")
        self.units = [(w, d, c) for w, d in ((self.wq_sb, self.qt),
                                             (self.wk_sb, self.kt))
                      for c in range(nch)]
        self.i = 0

    def step(self):
        if self.i >= len(self.units):
            return False
        w_sb, dst, c = self.units[self.i]
        self.i += 1
        nc = self.nc
        pp = self.psP.tile([128, 512], F32, tag="pp")
        for d in range(8):
            nc.tensor.matmul(pp, w_sb[:, d, :],
                             self.xt[:, c, d, :],
                             start=(d == 0), stop=(d == 7))
        nc.vector.tensor_copy(dst[:, c * 512:(c + 1) * 512], pp)
        return True

    def drain(self):
        while self.step():
            pass


class OutEmitter:
    """Output projection, dribbled into hp3's attention (hp3 runs its
    q-chunks descending, so l-tiles 12..4 project early); the last chunk's
    tiles 0-3 are split into an early v=0..2 partial plus a single-matmul
    finish so almost nothing runs after the final normalize."""

    def __init__(self, nc, pools, ct, wo_sb, out, drain_lts):
        self.nc = nc
        self.phco, self.psP = pools
        self.ct, self.wo_sb, self.out = ct, wo_sb, out
        self.drain_lts = drain_lts
        self.start = 0
        self.queue = []
        self.ost = {}
        self.pa = {}
        self.dq = 0

    def add_lt(self, lt):
        self.queue.extend([("full", lt, 0), ("full", lt, 1)])

    def add_partial(self, lt):
        self.queue.extend([("pA", lt, 0), ("pA", lt, 1)])

    def add_finish(self, lt):
        self.queue.extend([("fin", lt, 0), ("fin", lt, 1)])

    def step(self):
        if not self.queue:
            return False
        kind, lt, n = self.queue.pop(0)
        nc = self.nc
        pp = self.psP.tile([128, 512], F32, tag="pp", name="ppo")
        if kind == "pA":
            for v in range(3):
                nc.tensor.matmul(pp, self.ct[:, v, lt * 128:(lt + 1) * 128],
                                 self.wo_sb[:, v, n * 512:(n + 1) * 512],
                                 start=(v == 0), stop=(v == 2))
            pa = self.phco.tile([128, 512], F32, tag="pA", bufs=8,
                                name=f"pA{lt}_{n}")
            nc.vector.tensor_copy(pa, pp)
            self.pa[(lt, n)] = pa
            return True
        if n == 0:
            self.ost[lt] = self.phco.tile([128, D], BF16, tag="ost",
                                          name=f"ost{lt}")
        ost = self.ost[lt]
        if kind == "fin":
            nc.tensor.matmul(pp, self.ct[:, 3, lt * 128:(lt + 1) * 128],
                             self.wo_sb[:, 3, n * 512:(n + 1) * 512],
                             start=True, stop=True)
            nc.vector.tensor_add(ost[:, n * 512:(n + 1) * 512], pp,
                                 self.pa.pop((lt, n)))
            # final-tail halves: store immediately, alternating the sync
            # and ACT hw DGE queues (idle by now) so the stores overlap
            eng = nc.sync if self.dq % 2 == 0 else nc.scalar
            self.dq += 1
            eng.dma_start(
                out=self.out[lt * 128:(lt + 1) * 128,
                             n * 512:(n + 1) * 512],
                in_=ost[:, n * 512:(n + 1) * 512])
            if n == 1:
                del self.ost[lt]
            return True
        for v in range(4):
            nc.tensor.matmul(pp, self.ct[:, v, lt * 128:(lt + 1) * 128],
                             self.wo_sb[:, v, n * 512:(n + 1) * 512],
                             start=(v == 0), stop=(v == 3))
        nc.vector.tensor_copy(ost[:, n * 512:(n + 1) * 512], pp)
        if n == 1:
            # alternate the (idle) gpsimd and sync queues so the ~4MiB of
            # output stores drain in parallel instead of piling up
            eng = nc.gpsimd if lt % 2 == 0 else nc.sync
            eng.dma_start(out=self.out[lt * 128:(lt + 1) * 128, :],
                          in_=ost)
            del self.ost[lt]
        return True

    def drain(self):
        while self.step():
            pass


def build_nc(l=L):
    assert l % 512 == 0
    nch = l // 512           # q-chunks
    nlt = l // 128           # l-tiles
    nc = bacc.Bacc("TRN2", target_bir_lowering=False, debug=False,
                   num_devices=N_CORES)

    # all operands pre-shuffled on the host into partition-contiguous
    # DMA layouts (see make_in_maps)
    x = nc.dram_tensor("x", [nch, 128, 8, 512], BF16,
                       kind="ExternalInput").ap()
    wq = nc.dram_tensor("wq", [NHP, 128, 8, 128], BF16,
                        kind="ExternalInput").ap()
    wk = nc.dram_tensor("wk", [NHP, 128, 8, 128], BF16,
                        kind="ExternalInput").ap()
    wv = nc.dram_tensor("wv", [128, 8, OC], BF16, kind="ExternalInput").ap()
    wo = nc.dram_tensor("wo", [128, 4, D], BF16, kind="ExternalInput").ap()
    out = nc.dram_tensor("out", [l, D], BF16, kind="ExternalOutput").ap()

    with tile.TileContext(nc) as tc, ExitStack() as ctx:
        top = ctx.enter_context(tc.tile_pool(name="top", bufs=1))
        xtp = ctx.enter_context(tc.tile_pool(name="xtp", bufs=1))
        qkp = ctx.enter_context(tc.tile_pool(name="qkp", bufs=2))
        wp = ctx.enter_context(tc.tile_pool(name="wp", bufs=2))
        phco = ctx.enter_context(tc.tile_pool(name="phco", bufs=4))

        # V: [128(l), ltile, head, 65] - col 64 is ones (softmax denominator)
        vt = top.tile([128, nlt, HPC, DH + 1], BF16)
        ct = top.tile([128, NHP, l], BF16)        # normalized ctx^T
        tril = top.tile([128, 128], BF16)
        ones = top.tile([128, 1], F32)
        warm = top.tile([128, 1], BF16)
        onesw = top.tile([1, DV], BF16)           # rank-1 bcast weights
        xt = xtp.tile([128, nch, 8, 512], BF16)   # x^T, chunk-major

        wones = top.tile([128, 1], BF16)
        wbuf = top.tile([128, 256], BF16)

        nc.vector.memset(ones, 1.0)
        nc.vector.memset(onesw, 1.0)
        nc.vector.memset(wones, 1.0)
        nc.vector.memset(wbuf, 0.0)
        # warm-up exp: loads the ACT function table during the DMA-bound
        # startup instead of stalling the first attention group
        nc.scalar.activation(warm, ones, AF.Exp, scale=0.125)
        nc.vector.tensor_copy(
            vt[:, :, :, DV:DV + 1].rearrange("p t h c -> p (t h) c"),
            ones.broadcast_to((128, nlt * HPC, 1)))
        # causal keep-mask for S^T diag blocks: tril[k, q] = 1.0 iff q >= k
        nc.gpsimd.memset(tril, 0.0)
        nc.gpsimd.affine_select(
            out=tril, in_=tril, compare_op=mybir.AluOpType.is_gt,
            fill=1.0, base=0, pattern=[[-1, 128]], channel_multiplier=1)

        # ---------------- Prologue: xT DMA + V + QK(hp=0) -----------------
        with tc.tile_pool(name="wvp", bufs=1) as wvp, \
             tc.tile_pool(name="psPro", bufs=3, space="PSUM") as psPro:
            # DMA staging: the device DMA pipe is shared (8 cores pull
            # ~50MiB of inputs concurrently at kernel start, ~150-250 GB/s
            # effective per core), so ARRIVAL ORDER is everything. Transfers
            # on one queue run FIFO - putting all x chunks on sync in need
            # order stages them automatically; wv/wo ride scalar, Wq/Wk
            # gpsimd, so at most ~3 transfers share the pipe at once.
            wv_sb = wvp.tile([128, 8, OC], BF16)
            nc.scalar.dma_start(out=wv_sb[:, 0:4, :], in_=wv[:, 0:4, :])
            nc.sync.dma_start(out=xt[:, 0, :, 0:256], in_=x[0, :, :, 0:256])
            nc.scalar.dma_start(out=wv_sb[:, 4:8, :], in_=wv[:, 4:8, :])
            nc.sync.dma_start(out=xt[:, 0, :, 256:512],
                              in_=x[0, :, :, 256:512])
            for c in range(1, nch):
                nc.sync.dma_start(out=xt[:, c], in_=x[c])
            # ~24 rank-1 warm-up matmuls fill the DMA wait with PE activity
            # so the HAM clock gate reaches 8/8 before the first real matmul
            for w in range(24):
                pw = psPro.tile([1, 256], F32, tag="pwarm", bufs=2)
                nc.tensor.matmul(pw, wones, wbuf, start=True, stop=True)
            for c in range(nch):
                if c == 1:
                    # hp0's Wq/Wk ride the (idle) gpsimd queue
                    em = ProjEmitter(nc, 0, (qkp, wp, psPro), xt, wq, wk, nch)
                # V for this l-chunk
                for m in range(4):
                    pp = psPro.tile([128, 512], F32, tag="pp")
                    for d in range(8):
                        nc.tensor.matmul(
                            pp, xt[:, c, d, m * 128:(m + 1) * 128],
                            wv_sb[:, d, :], start=(d == 0), stop=(d == 7))
                    nc.vector.tensor_copy(
                        vt[:, c * 4 + m, :, 0:DV],
                        pp.rearrange("p (h v) -> p h v", h=HPC))
            em.drain()

        # Prefetch Wo now: the scalar DMA queue drains during early hp0
        # attention, long before the output projection needs it.
        phc = ctx.enter_context(tc.tile_pool(name="phc", bufs=1))
        wo_sb = phc.tile([128, 4, D], BF16)
        nc.scalar.dma_start(out=wo_sb, in_=wo)

        # ---------------- Main: attention + next-pair projections ---------
        with tc.tile_pool(name="phb", bufs=2) as phb, \
             tc.tile_pool(name="psS", bufs=2, space="PSUM") as psS, \
             tc.tile_pool(name="psPd", bufs=2, space="PSUM") as psPd, \
             tc.tile_pool(name="psC", bufs=2, space="PSUM") as psC:
            n_groups_hp = 2 * nch * (nch + 1)
            for hp in range(NHP):
                qt, kt = em.qt, em.kt
                if hp + 1 < NHP:
                    # hp0 runs j descending, so its shallow (bubble-prone)
                    # chunks come last: dribble the projection units there.
                    # Ascending hps dribble early (after the Wq/Wk DMA
                    # lands) into their shallow first chunks.
                    em = ProjEmitter(nc, hp + 1, (qkp, wp, psPd), xt, wq, wk,
                                     nch, start=24 if hp == 0 else 4)
                    cadence = 1
                else:
                    em = OutEmitter(nc, (phco, psPd), ct, wo_sb, out,
                                    drain_lts=range(0, 4))
                    # lt 0-3's head-pair 0-2 ctx has been ready since hp2:
                    # dribble their v=0..2 partials during hp3's first chunk
                    for lt in range(4):
                        em.add_partial(lt)
                    cadence = 1
                gcount = 0

                def pv_step(g, j, pctx, pexp, po, H):
                    # masks + PV for group g (one group after its exp)
                    for r2 in range(2):
                        kt_i = 2 * g + r2
                        r = kt_i - 4 * j
                        c0 = 0
                        if r >= 0:      # diagonal k-tile
                            c0 = r * 128
                            nc.vector.tensor_mul(
                                pexp[:, r2, c0:c0 + 128],
                                pexp[:, r2, c0:c0 + 128], tril)
                        nc.tensor.matmul(
                            pctx[:, c0:512],
                            vt[:, kt_i, H, :],
                            pexp[:, r2, c0:512],
                            start=(kt_i == 0), stop=(kt_i == 4 * j + 3))

                # hp0 and hp3 run j descending: hp0 so j=3's 32 S matmuls
                # hide the cold-ACT exp latency at the prologue boundary,
                # hp3 so the big l-tiles' out-projection dribbles early and
                # only lt 0-3 remain for the drain.
                desc = hp == 0 or hp == NHP - 1
                jorder = range(nch - 1, -1, -1) if desc else range(nch)
                last_j = 0 if desc else nch - 1
                for j in jorder:
                    n_g = 2 * (j + 1)
                    pctxs = {}
                    prevs = {0: None, 1: None}
                    for h in range(2):
                        pctxs[h] = psC.tile([DV + 1, 512], F32,
                                            tag="pctx", name=f"pctx{h}")
                    for g in range(n_g + 1):
                        # S^T for both heads interleaved r2-major: head0's
                        # matmuls run on PE rows 0-63 (tile T0), head1's on
                        # rows 64-127 (T8); with no full-array matmul
                        # between them the two 64x128 tiles overlap.
                        pscs = {}
                        if g < n_g:
                            for h in range(2):
                                pscs[h] = psS.tile([128, 2, 512], F32,
                                                   tag="psc", name=f"psc{h}")
                            for r2 in range(2):
                                kt_i = 2 * g + r2
                                c0 = max(0, kt_i - 4 * j) * 128
                                for h in range(2):
                                    po = 64 * h
                                    nc.tensor.matmul(
                                        pscs[h][:, r2, c0:512],
                                        kt[po:po + DH,
                                           kt_i * 128:(kt_i + 1) * 128],
                                        qt[po:po + DH,
                                           j * 512 + c0:(j + 1) * 512],
                                        start=True, stop=True)
                            for h in range(2):
                                pexp = phb.tile([128, 2, 512], BF16,
                                                tag="pexp", bufs=10,
                                                name=f"pexp{h}")
                                cg = max(0, 2 * g - 4 * j) * 128
                                nc.scalar.activation(pexp[:, :, cg:512],
                                                     pscs[h][:, :, cg:512],
                                                     AF.Exp, scale=0.125)
                                pscs[h] = pexp
                        for h in range(2):
                            po = 64 * h
                            H = 2 * hp + h
                            if prevs[h] is not None:
                                pv_step(prevs[h][0], j, pctxs[h],
                                        prevs[h][1], po, H)
                                gcount += 1
                                if (em and cadence and gcount > em.start
                                        and gcount % cadence == 0):
                                    em.step()
                            prevs[h] = (g, pscs[h]) if g < n_g else None
                    if hp == NHP - 1 and j == last_j:
                        # flush remaining independent out-proj units BEFORE
                        # the final normalize: the normalize-dependent rank-1
                        # broadcasts otherwise block them at the head of the
                        # strict-FIFO PE queue (and the idle re-throttles the
                        # HAM clock gate, running the whole drain at 1.2GHz)
                        em.drain()
                        # final chunk, both heads' chains interleaved:
                        # broadcast via rank-1 PE matmuls (~0.2us) instead
                        # of the ~1us gpsimd path; recip reads PSUM directly
                        # and emits bf16 so the chain is 3 DVE ops deep
                        ctus, invbs, bcps = {}, {}, {}
                        rss, invs = {}, {}
                        for h in range(2):
                            # reciprocal_approx is a bit-trick op: stage the
                            # PSUM denominator row to SBUF (IEEE f32) first
                            rss[h] = phb.tile([1, 512], F32, tag="rs",
                                              name=f"rs{h}")
                            nc.vector.tensor_copy(rss[h],
                                                  pctxs[h][DV:DV + 1, :])
                        for h in range(2):
                            ctus[h] = phb.tile([64, 512], BF16, tag="ctu",
                                               name=f"ctu{h}")
                            nc.vector.tensor_copy(ctus[h], pctxs[h][0:DV, :])
                        for h in range(2):
                            invs[h] = phb.tile([1, 512], F32, tag="inv",
                                               name=f"inv{h}")
                            nc.vector.reciprocal_approx_fast(
                                out=invs[h], in_=rss[h])
                        for h in range(2):
                            invbs[h] = phb.tile([1, 512], BF16, tag="invb",
                                                name=f"invb{h}")
                            nc.vector.tensor_copy(invbs[h], invs[h])
                        for h in range(2):
                            po = 64 * h
                            bcp = psPd.tile([128, 512], F32, tag="pp",
                                            name=f"bcp{h}")
                            nc.tensor.matmul(bcp[0:DV, :], onesw,
                                             invbs[h], start=True, stop=True)
                            nc.vector.tensor_mul(
                                ct[po:po + DV, hp, j * 512:(j + 1) * 512],
                                ctus[h], bcp[0:DV, :])
                    else:
                        for h in range(2):
                            po = 64 * h
                            # free the pctx PSUM bank after one DVE copy +
                            # a PSUM-direct recip (the next j's first PV
                            # waits on it); broadcast/scale dangle off the
                            # critical path.
                            rs = phb.tile([1, 512], F32, tag="rs",
                                          name="rs")
                            nc.vector.tensor_copy(rs,
                                                  pctxs[h][DV:DV + 1, :])
                            ctu = phb.tile([64, 512], BF16, tag="ctu",
                                           name=f"ctu{h}")
                            nc.vector.tensor_copy(ctu, pctxs[h][0:DV, :])
                            inv = phb.tile([1, 512], F32, tag="inv",
                                           name="inv")
                            nc.vector.reciprocal_approx_fast(
                                out=inv, in_=rs)
                            bc = phb.tile([64, 512], F32, tag="bc",
                                          name="bc")
                            nc.gpsimd.partition_broadcast(out_ap=bc,
                                                          in_ap=inv)
                            nc.vector.tensor_mul(
                                ct[po:po + DV, hp, j * 512:(j + 1) * 512],
                                ctu, bc)
                    if hp == NHP - 1:
                        if j == last_j:
                            em.add_finish(0)
                            em.add_finish(1)
                            em.add_finish(2)
                            em.add_finish(3)
                        else:
                            for lt in range(4 * j, 4 * j + 4):
                                em.add_lt(lt)
                em.drain()

    nc.compile()
    return nc


def _bf16(a):
    import ml_dtypes
    return np.ascontiguousarray(a).astype(ml_dtypes.bfloat16)


def make_in_maps(x, Wq, Wk, Wv, Wo):
    nch = L // 512
    in_maps = []
    for c in range(N_CORES):
        b, g = c // 2, c % 2
        # x^T chunk-major: [c, p, t, lc] = x[b][c*512+lc, t*128+p]
        xdev = x[b].reshape(nch, 512, 8, 128).transpose(0, 3, 2, 1)
        wqg = Wq[:, g * OC:(g + 1) * OC]
        wkg = Wk[:, g * OC:(g + 1) * OC]
        # per-head-pair [hp, p, t, o] = W[t*128+p, hp*128+o]
        wqd = wqg.reshape(8, 128, NHP, 128).transpose(2, 1, 0, 3)
        wkd = wkg.reshape(8, 128, NHP, 128).transpose(2, 1, 0, 3)
        # [p, t, o] = W[t*128+p, o]
        wvd = Wv[:, g * OC:(g + 1) * OC].reshape(8, 128, OC).transpose(1, 0, 2)
        # [p, v, o] = Wo[v*128+p, o]
        wod = Wo[g * OC:(g + 1) * OC, :].reshape(4, 128, D).transpose(1, 0, 2)
        in_maps.append({
            "x": _bf16(xdev),
            "wq": _bf16(wqd),
            "wk": _bf16(wkd),
            "wv": _bf16(wvd),
            "wo": _bf16(wod),
        })
    return in_maps


_NC_CACHE = {}


def _get_nc():
    if "nc" not in _NC_CACHE:
        _NC_CACHE["nc"] = build_nc()
    return _NC_CACHE["nc"]


def _numpy_fallback(x, Wq, Wk, Wv, Wo, bo, mask):
    Bsz, Lq, _ = x.shape
    Q = (x @ Wq).reshape(Bsz, Lq, N_HEAD, DH).transpose(0, 2, 1, 3)
    K = (x @ Wk).reshape(Bsz, Lq, N_HEAD, DH).transpose(0, 2, 1, 3)
    V = (x @ Wv).reshape(Bsz, Lq, N_HEAD, DV).transpose(0, 2, 1, 3)
    s = np.einsum("bhqd,bhkd->bhqk", Q, K) / np.sqrt(np.float32(DH))
    s = np.where(mask, s, -np.inf)
    s = s - s.max(axis=-1, keepdims=True)
    p = np.exp(s)
    p /= p.sum(axis=-1, keepdims=True)
    ctxv = np.einsum("bhqk,bhkv->bhqv", p, V)
    ctxv = ctxv.transpose(0, 2, 1, 3).reshape(Bsz, Lq, N_HEAD * DV)
    return (ctxv @ Wo + bo).astype(np.float32)


def run_on_hw(in_maps, trace=False):
    from concourse.bass_utils import run_bass_kernel_spmd
    nc = _get_nc()
    return run_bass_kernel_spmd(nc, in_maps, list(range(N_CORES)), trace=trace)


def kernel(x, Wq, Wk, Wv, Wo, bo, mask, _trace=False, _results=None):
    x = np.asarray(x, dtype=np.float32)
    Wq = np.asarray(Wq, dtype=np.float32)
    Wk = np.asarray(Wk, dtype=np.float32)
    Wv = np.asarray(Wv, dtype=np.float32)
    Wo = np.asarray(Wo, dtype=np.float32)
    bo = np.asarray(bo, dtype=np.float32)
    mask_np = np.asarray(mask).reshape(mask.shape[-2], mask.shape[-1])

    causal = bool(np.array_equal(
        mask_np, np.tril(np.ones((L, L), dtype=bool))))
    if not causal or x.shape != (B, L, D):
        return _numpy_fallback(np.asarray(x), Wq, Wk, Wv, Wo, bo,
                               np.asarray(mask))

    res = run_on_hw(make_in_maps(x, Wq, Wk, Wv, Wo), trace=_trace)
    if _results is not None:
        _results.append(res)
    out = np.empty((B, L, D), dtype=np.float32)
    for b in range(B):
        out[b] = (np.asarray(res.results[2 * b]["out"], dtype=np.float32)
                  + np.asarray(res.results[2 * b + 1]["out"], dtype=np.float32)
                  + bo)
    return out


# revision 27
# speedup vs baseline: 1.1064x; 1.0496x over previous
"""Causal multi-head attention Trainium2 kernel (8 NeuronCores).

Problem: B=4, L=2048, D=1024, 16 heads x (dh=64, dv=64), causal mask.
Sharding: data-parallel over batch (4) x tensor-parallel over heads (2 groups
of 8). Core c handles batch c//2, head-group c%2. Each core computes its
partial output projection (ctx_g @ Wo_g); the host sums the two head-group
partials per batch and adds the bias.

v2-v4 (~322us): software-pipelined flash-style attention; S^T tiles = K@Q^T;
exp on ACT (scale=1/8 folded in); causal diagonal via tril tensor_mul +
width-restricted matmuls; ones column of V_aug gives the softmax denominator
in PSUM row 64; bf16 datapath; projections and output projection dribbled
into the attention stream.

v5 (~309us): x arrives pre-transposed from the host; the two heads' S^T
matmuls (contraction 64: head0 on PE rows 0-63/tile T0, head1 rows 64-127/
T8) issue adjacently so the hardware overlaps them on the two 64x128 PE
tiles.

v6: (a) all DRAM operands pre-shuffled on the host into partition-contiguous
layouts (x chunk-major [nch,128,8,512], weights [128,t,o]) so the input DMAs
run at full HBM bandwidth - v5 lost ~15us waiting on 1KB-segment descriptor
patterns at startup. (b) hp3 processes q-chunks descending: the large
l-tiles' output projections dribble into the remaining attention, and the
final drain is only lt 0-3, split into an early v=0..2 partial (pA, dribbled)
plus a 1-matmul finish - the v5 drain ran ~23 matmuls after a >3us PE gap
that re-throttled the HAM clock gate to 1.2 GHz. (c) output stored as bf16
(halves the store bytes; host upcasts and sums the TP partials in f32).
(d) Wq/Wk loads ride the gpsimd queue - their engine-blocking DIRECT2D
triggers were stalling the exp stream on the ACT queue. (e) S quartet issue
order alternates heads (h0r0,h1r0,h0r1,h1r1) for tile overlap.
"""

import numpy as np
from contextlib import ExitStack

import concourse.bass as bass
import concourse.tile as tile
from concourse import bacc, mybir

F32 = mybir.dt.float32
BF16 = mybir.dt.bfloat16
AF = mybir.ActivationFunctionType

B, L, D = 4, 2048, 1024
N_HEAD, DH, DV = 16, 64, 64
N_CORES = 8
HPC = N_HEAD // 2          # heads per core (8)
OC = HPC * DH              # per-core projection width (512)
NHP = HPC // 2             # head-pairs per core (4)


class ProjEmitter:
    """Q^T/K^T projection for one head-pair, emitted in per-(proj,chunk)
    units so the matmuls interleave with attention of the previous pair."""

    def __init__(self, nc, hp, pools, xt, wq, wk, nch, start=0):
        self.nc = nc
        self.xt = xt
        self.start = start
        qkp, wp, self.psP = pools
        self.wq_sb = wp.tile([128, 8, 128], BF16, tag="wq")
        self.wk_sb = wp.tile([128, 8, 128], BF16, tag="wk")
        # gpsimd queue: keeps the engine-blocking DIRECT2D weight triggers
        # off the ACT queue (exps) and the sync queue (normalize bcasts)
        nc.gpsimd.dma_start(out=self.wq_sb, in_=wq[hp])
        nc.gpsimd.dma_start(out=self.wk_sb, in_=wk[hp])
        self.qt = qkp.tile([128, nch * 512], BF16, tag="qt")
        self.kt = qkp.tile([128, nch * 512], BF16, tag="kt")
        self.units = [(w, d, c) for w, d in ((self.wq_sb, self.qt),
                                             (self.wk_sb, self.kt))
                      for c in range(nch)]
        self.i = 0

    def step(self):
        if self.i >= len(self.units):
            return False
        w_sb, dst, c = self.units[self.i]
        self.i += 1
        nc = self.nc
        pp = self.psP.tile([128, 512], F32, tag="pp")
        for d in range(8):
            nc.tensor.matmul(pp, w_sb[:, d, :],
                             self.xt[:, c, d, :],
                             start=(d == 0), stop=(d == 7))
        nc.vector.tensor_copy(dst[:, c * 512:(c + 1) * 512], pp)
        return True

    def drain(self):
        while self.step():
            pass


class OutEmitter:
    """Output projection, dribbled into hp3's attention (hp3 runs its
    q-chunks descending, so l-tiles 12..4 project early); the last chunk's
    tiles 0-3 are split into an early v=0..2 partial plus a single-matmul
    finish so almost nothing runs after the final normalize."""

    def __init__(self, nc, pools, ct, wo_sb, out, drain_lts):
        self.nc = nc
        self.phco, self.psP = pools
        self.ct, self.wo_sb, self.out = ct, wo_sb, out
        self.drain_lts = drain_lts
        self.start = 0
        self.queue = []
        self.ost = {}
        self.pa = {}
        self.dq = 0

    def add_lt(self, lt):
        self.queue.extend([("full", lt, 0), ("full", lt, 1)])

    def add_partial(self, lt):
        self.queue.extend([("pA", lt, 0), ("pA", lt, 1)])

    def add_finish(self, lt):
        self.queue.extend([("fin", lt, 0), ("fin", lt, 1)])

    def step(self):
        if not self.queue:
            return False
        kind, lt, n = self.queue.pop(0)
        nc = self.nc
        pp = self.psP.tile([128, 512], F32, tag="pp", name="ppo")
        if kind == "pA":
            for v in range(3):
                nc.tensor.matmul(pp, self.ct[:, v, lt * 128:(lt + 1) * 128],
                                 self.wo_sb[:, v, n * 512:(n + 1) * 512],
                                 start=(v == 0), stop=(v == 2))
            pa = self.phco.tile([128, 512], F32, tag="pA", bufs=8,
                                name=f"pA{lt}_{n}")
            nc.vector.tensor_copy(pa, pp)
            self.pa[(lt, n)] = pa
            return True
        if n == 0:
            self.ost[lt] = self.phco.tile([128, D], BF16, tag="ost",
                                          name=f"ost{lt}")
        ost = self.ost[lt]
        if kind == "fin":
            nc.tensor.matmul(pp, self.ct[:, 3, lt * 128:(lt + 1) * 128],
                             self.wo_sb[:, 3, n * 512:(n + 1) * 512],
                             start=True, stop=True)
            nc.vector.tensor_add(ost[:, n * 512:(n + 1) * 512], pp,
                                 self.pa.pop((lt, n)))
            # final-tail halves: store immediately, alternating the sync
            # and ACT hw DGE queues (idle by now) so the stores overlap
            eng = nc.sync if self.dq % 2 == 0 else nc.scalar
            self.dq += 1
            eng.dma_start(
                out=self.out[lt * 128:(lt + 1) * 128,
                             n * 512:(n + 1) * 512],
                in_=ost[:, n * 512:(n + 1) * 512])
            if n == 1:
                del self.ost[lt]
            return True
        for v in range(4):
            nc.tensor.matmul(pp, self.ct[:, v, lt * 128:(lt + 1) * 128],
                             self.wo_sb[:, v, n * 512:(n + 1) * 512],
                             start=(v == 0), stop=(v == 3))
        nc.vector.tensor_copy(ost[:, n * 512:(n + 1) * 512], pp)
        if n == 1:
            # alternate the (idle) gpsimd and sync queues so the ~4MiB of
            # output stores drain in parallel instead of piling up
            eng = nc.gpsimd if lt % 2 == 0 else nc.sync
            eng.dma_start(out=self.out[lt * 128:(lt + 1) * 128, :],
                          in_=ost)
            del self.ost[lt]
        return True

    def drain(self):
        while self.step():
            pass


def build_nc(l=L):
    assert l % 512 == 0
    nch = l // 512           # q-chunks
    nlt = l // 128           # l-tiles
    nc = bacc.Bacc("TRN2", target_bir_lowering=False, debug=False,
                   num_devices=N_CORES)

    # all operands pre-shuffled on the host into partition-contiguous
    # DMA layouts (see make_in_maps)
    x = nc.dram_tensor("x", [nch, 128, 8, 512], BF16,
                       kind="ExternalInput").ap()
    wq = nc.dram_tensor("wq", [NHP, 128, 8, 128], BF16,
                        kind="ExternalInput").ap()
    wk = nc.dram_tensor("wk", [NHP, 128, 8, 128], BF16,
                        kind="ExternalInput").ap()
    wv = nc.dram_tensor("wv", [128, 8, OC], BF16, kind="ExternalInput").ap()
    wo = nc.dram_tensor("wo", [128, 4, D], BF16, kind="ExternalInput").ap()
    out = nc.dram_tensor("out", [l, D], BF16, kind="ExternalOutput").ap()

    with tile.TileContext(nc) as tc, ExitStack() as ctx:
        top = ctx.enter_context(tc.tile_pool(name="top", bufs=1))
        xtp = ctx.enter_context(tc.tile_pool(name="xtp", bufs=1))
        qkp = ctx.enter_context(tc.tile_pool(name="qkp", bufs=2))
        wp = ctx.enter_context(tc.tile_pool(name="wp", bufs=2))
        phco = ctx.enter_context(tc.tile_pool(name="phco", bufs=4))

        # V: [128(l), ltile, head, 65] - col 64 is ones (softmax denominator)
        vt = top.tile([128, nlt, HPC, DH + 1], BF16)
        ct = top.tile([128, NHP, l], BF16)        # normalized ctx^T
        tril = top.tile([128, 128], BF16)
        ones = top.tile([128, 1], F32)
        warm = top.tile([128, 1], BF16)
        onesw = top.tile([1, DV], BF16)           # rank-1 bcast weights
        xt = xtp.tile([128, nch, 8, 512], BF16)   # x^T, chunk-major

        wones = top.tile([128, 1], BF16)
        wbuf = top.tile([128, 256], BF16)

        nc.vector.memset(ones, 1.0)
        nc.vector.memset(onesw, 1.0)
        nc.vector.memset(wones, 1.0)
        nc.vector.memset(wbuf, 0.0)
        # warm-up exp: loads the ACT function table during the DMA-bound
        # startup instead of stalling the first attention group
        nc.scalar.activation(warm, ones, AF.Exp, scale=0.125)
        nc.vector.tensor_copy(
            vt[:, :, :, DV:DV + 1].rearrange("p t h c -> p (t h) c"),
            ones.broadcast_to((128, nlt * HPC, 1)))
        # causal keep-mask for S^T diag blocks: tril[k, q] = 1.0 iff q >= k
        nc.gpsimd.memset(tril, 0.0)
        nc.gpsimd.affine_select(
            out=tril, in_=tril, compare_op=mybir.AluOpType.is_gt,
            fill=1.0, base=0, pattern=[[-1, 128]], channel_multiplier=1)

        # ---------------- Prologue: xT DMA + V + QK(hp=0) -----------------
        with tc.tile_pool(name="wvp", bufs=1) as wvp, \
             tc.tile_pool(name="psPro", bufs=3, space="PSUM") as psPro:
            # DMA staging: the device DMA pipe is shared (8 cores pull
            # ~50MiB of inputs concurrently at kernel start, ~150-250 GB/s
            # effective per core), so ARRIVAL ORDER is everything. Transfers
            # on one queue run FIFO - putting all x chunks on sync in need
            # order stages them automatically; wv/wo ride scalar, Wq/Wk
            # gpsimd, so at most ~3 transfers share the pipe at once.
            wv_sb = wvp.tile([128, 8, OC], BF16)
            nc.scalar.dma_start(out=wv_sb[:, 0:4, :], in_=wv[:, 0:4, :])
            nc.sync.dma_start(out=xt[:, 0, :, 0:256], in_=x[0, :, :, 0:256])
            nc.scalar.dma_start(out=wv_sb[:, 4:8, :], in_=wv[:, 4:8, :])
            nc.sync.dma_start(out=xt[:, 0, :, 256:512],
                              in_=x[0, :, :, 256:512])
            for c in range(1, nch):
                nc.sync.dma_start(out=xt[:, c], in_=x[c])
            # ~24 rank-1 warm-up matmuls fill the DMA wait with PE activity
            # so the HAM clock gate reaches 8/8 before the first real matmul
            for w in range(24):
                pw = psPro.tile([1, 256], F32, tag="pwarm", bufs=2)
                nc.tensor.matmul(pw, wones, wbuf, start=True, stop=True)
            for c in range(nch):
                if c == 1:
                    # hp0's Wq/Wk ride the (idle) gpsimd queue
                    em = ProjEmitter(nc, 0, (qkp, wp, psPro), xt, wq, wk, nch)
                # V for this l-chunk
                for m in range(4):
                    pp = psPro.tile([128, 512], F32, tag="pp")
                    for d in range(8):
                        nc.tensor.matmul(
                            pp, xt[:, c, d, m * 128:(m + 1) * 128],
                            wv_sb[:, d, :], start=(d == 0), stop=(d == 7))
                    nc.vector.tensor_copy(
                        vt[:, c * 4 + m, :, 0:DV],
                        pp.rearrange("p (h v) -> p h v", h=HPC))
            em.drain()

        # Prefetch Wo now: the scalar DMA queue drains during early hp0
        # attention, long before the output projection needs it.
        phc = ctx.enter_context(tc.tile_pool(name="phc", bufs=1))
        wo_sb = phc.tile([128, 4, D], BF16)
        nc.scalar.dma_start(out=wo_sb, in_=wo)

        # ---------------- Main: attention + next-pair projections ---------
        with tc.tile_pool(name="phb", bufs=2) as phb, \
             tc.tile_pool(name="psS", bufs=2, space="PSUM") as psS, \
             tc.tile_pool(name="psPd", bufs=2, space="PSUM") as psPd, \
             tc.tile_pool(name="psC", bufs=2, space="PSUM") as psC:
            n_groups_hp = 2 * nch * (nch + 1)
            for hp in range(NHP):
                qt, kt = em.qt, em.kt
                if hp + 1 < NHP:
                    # hp0 runs j descending, so its shallow (bubble-prone)
                    # chunks come last: dribble the projection units there.
                    # Ascending hps dribble early (after the Wq/Wk DMA
                    # lands) into their shallow first chunks.
                    em = ProjEmitter(nc, hp + 1, (qkp, wp, psPd), xt, wq, wk,
                                     nch, start=24 if hp == 0 else 4)
                    cadence = 1
                else:
                    em = OutEmitter(nc, (phco, psPd), ct, wo_sb, out,
                                    drain_lts=range(0, 4))
                    # lt 0-3's head-pair 0-2 ctx has been ready since hp2:
                    # dribble their v=0..2 partials during hp3's first chunk
                    for lt in range(4):
                        em.add_partial(lt)
                    cadence = 1
                gcount = 0

                # hp0 and hp3 run j descending: hp0 so j=3's 32 S matmuls
                # hide the cold-ACT exp latency at the prologue boundary,
                # hp3 so the big l-tiles' out-projection dribbles early and
                # only lt 0-3 remain for the drain.
                desc = hp == 0 or hp == NHP - 1
                jorder = range(nch - 1, -1, -1) if desc else range(nch)
                last_j = 0 if desc else nch - 1
                for j in jorder:
                    n_g = 2 * (j + 1)
                    pctxs = {}
                    prev = None
                    for h in range(2):
                        pctxs[h] = psC.tile([DV + 1, 512], F32,
                                            tag="pctx", name=f"pctx{h}")
                    for g in range(n_g + 1):
                        # k-tile-major psc tiles [128, head, q]: the two
                        # heads' S^T matmuls per k-tile issue back-to-back
                        # (head0 on PE rows 0-63/tile T0, head1 on rows
                        # 64-127/T8) and overlap on the two 64x128 PE
                        # tiles; each k-tile's exp covers exactly its
                        # causal width (no garbage columns), and the psc
                        # ring gates the next group's first S pair on the
                        # EARLY exp of this group.
                        pexps = {}
                        if g < n_g:
                            for r2 in range(2):
                                kt_i = 2 * g + r2
                                c0 = max(0, kt_i - 4 * j) * 128
                                psc = psS.tile([128, 2, 512], F32,
                                               tag="psc", name=f"psc{r2}")
                                for h in range(2):
                                    po = 64 * h
                                    nc.tensor.matmul(
                                        psc[:, h, c0:512],
                                        kt[po:po + DH,
                                           kt_i * 128:(kt_i + 1) * 128],
                                        qt[po:po + DH,
                                           j * 512 + c0:(j + 1) * 512],
                                        start=True, stop=True)
                                pexp = phb.tile([128, 2, 512], BF16,
                                                tag="pexp", bufs=10,
                                                name=f"pexp{r2}")
                                nc.scalar.activation(pexp[:, :, c0:512],
                                                     psc[:, :, c0:512],
                                                     AF.Exp, scale=0.125)
                                pexps[r2] = (pexp, c0)
                        if prev is not None:
                            pg, ppexps = prev
                            # causal keep-mask on diagonal k-tiles, both
                            # heads in one multiply
                            for r2 in range(2):
                                kt_i = 2 * pg + r2
                                if kt_i - 4 * j >= 0:
                                    c0 = (kt_i - 4 * j) * 128
                                    pexp = ppexps[r2][0]
                                    nc.vector.tensor_mul(
                                        pexp[:, :, c0:c0 + 128],
                                        pexp[:, :, c0:c0 + 128],
                                        tril.rearrange("p (o q) -> p o q",
                                                       o=1)
                                        .broadcast_to((128, 2, 128)))
                            for h in range(2):
                                H = 2 * hp + h
                                for r2 in range(2):
                                    kt_i = 2 * pg + r2
                                    pexp, c0 = ppexps[r2]
                                    nc.tensor.matmul(
                                        pctxs[h][:, c0:512],
                                        vt[:, kt_i, H, :],
                                        pexp[:, h, c0:512],
                                        start=(kt_i == 0),
                                        stop=(kt_i == 4 * j + 3))
                                gcount += 1
                                if (em and cadence and gcount > em.start
                                        and gcount % cadence == 0):
                                    em.step()
                        prev = (g, pexps) if g < n_g else None
                    if hp == NHP - 1 and j == last_j:
                        # flush remaining independent out-proj units BEFORE
                        # the final normalize: the normalize-dependent rank-1
                        # broadcasts otherwise block them at the head of the
                        # strict-FIFO PE queue (and the idle re-throttles the
                        # HAM clock gate, running the whole drain at 1.2GHz)
                        em.drain()
                        # final chunk, both heads' chains interleaved:
                        # broadcast via rank-1 PE matmuls (~0.2us) instead
                        # of the ~1us gpsimd path; recip reads PSUM directly
                        # and emits bf16 so the chain is 3 DVE ops deep
                        ctus, invbs, bcps = {}, {}, {}
                        rss, invs = {}, {}
                        for h in range(2):
                            # reciprocal_approx is a bit-trick op: stage the
                            # PSUM denominator row to SBUF (IEEE f32) first
                            rss[h] = phb.tile([1, 512], F32, tag="rs",
                                              name=f"rs{h}")
                            nc.vector.tensor_copy(rss[h],
                                                  pctxs[h][DV:DV + 1, :])
                        for h in range(2):
                            ctus[h] = phb.tile([64, 512], BF16, tag="ctu",
                                               name=f"ctu{h}")
                            nc.vector.tensor_copy(ctus[h], pctxs[h][0:DV, :])
                        for h in range(2):
                            invs[h] = phb.tile([1, 512], F32, tag="inv",
                                               name=f"inv{h}")
                            nc.vector.reciprocal_approx_fast(
                                out=invs[h], in_=rss[h])
                        for h in range(2):
                            invbs[h] = phb.tile([1, 512], BF16, tag="invb",
                                                name=f"invb{h}")
                            nc.vector.tensor_copy(invbs[h], invs[h])
                        for h in range(2):
                            po = 64 * h
                            bcp = psPd.tile([128, 512], F32, tag="pp",
                                            name=f"bcp{h}")
                            nc.tensor.matmul(bcp[0:DV, :], onesw,
                                             invbs[h], start=True, stop=True)
                            nc.vector.tensor_mul(
                                ct[po:po + DV, hp, j * 512:(j + 1) * 512],
                                ctus[h], bcp[0:DV, :])
                    else:
                        for h in range(2):
                            po = 64 * h
                            # free the pctx PSUM bank after one DVE copy +
                            # a PSUM-direct recip (the next j's first PV
                            # waits on it); broadcast/scale dangle off the
                            # critical path.
                            rs = phb.tile([1, 512], F32, tag="rs",
                                          name="rs")
                            nc.vector.tensor_copy(rs,
                                                  pctxs[h][DV:DV + 1, :])
                            ctu = phb.tile([64, 512], BF16, tag="ctu",
                                           name=f"ctu{h}")
                            nc.vector.tensor_copy(ctu, pctxs[h][0:DV, :])
                            inv = phb.tile([1, 512], F32, tag="inv",
                                           name="inv")
                            nc.vector.reciprocal_approx_fast(
                                out=inv, in_=rs)
                            bc = phb.tile([64, 512], F32, tag="bc",
                                          name="bc")
                            nc.gpsimd.partition_broadcast(out_ap=bc,
                                                          in_ap=inv)
                            nc.vector.tensor_mul(
                                ct[po:po + DV, hp, j * 512:(j + 1) * 512],
                                ctu, bc)
                    if hp == NHP - 1:
                        if j == last_j:
                            em.add_finish(0)
                            em.add_finish(1)
                            em.add_finish(2)
                            em.add_finish(3)
                        else:
                            for lt in range(4 * j, 4 * j + 4):
                                em.add_lt(lt)
                em.drain()

    nc.compile()
    return nc


def _bf16(a):
    import ml_dtypes
    return np.ascontiguousarray(a).astype(ml_dtypes.bfloat16)


def make_in_maps(x, Wq, Wk, Wv, Wo):
    nch = L // 512
    in_maps = []
    for c in range(N_CORES):
        b, g = c // 2, c % 2
        # x^T chunk-major: [c, p, t, lc] = x[b][c*512+lc, t*128+p]
        xdev = x[b].reshape(nch, 512, 8, 128).transpose(0, 3, 2, 1)
        wqg = Wq[:, g * OC:(g + 1) * OC]
        wkg = Wk[:, g * OC:(g + 1) * OC]
        # per-head-pair [hp, p, t, o] = W[t*128+p, hp*128+o]
        wqd = wqg.reshape(8, 128, NHP, 128).transpose(2, 1, 0, 3)
        wkd = wkg.reshape(8, 128, NHP, 128).transpose(2, 1, 0, 3)
        # [p, t, o] = W[t*128+p, o]
        wvd = Wv[:, g * OC:(g + 1) * OC].reshape(8, 128, OC).transpose(1, 0, 2)
        # [p, v, o] = Wo[v*128+p, o]
        wod = Wo[g * OC:(g + 1) * OC, :].reshape(4, 128, D).transpose(1, 0, 2)
        in_maps.append({
            "x": _bf16(xdev),
            "wq": _bf16(wqd),
            "wk": _bf16(wkd),
            "wv": _bf16(wvd),
            "wo": _bf16(wod),
        })
    return in_maps


_NC_CACHE = {}


def _get_nc():
    if "nc" not in _NC_CACHE:
        _NC_CACHE["nc"] = build_nc()
    return _NC_CACHE["nc"]


def _numpy_fallback(x, Wq, Wk, Wv, Wo, bo, mask):
    Bsz, Lq, _ = x.shape
    Q = (x @ Wq).reshape(Bsz, Lq, N_HEAD, DH).transpose(0, 2, 1, 3)
    K = (x @ Wk).reshape(Bsz, Lq, N_HEAD, DH).transpose(0, 2, 1, 3)
    V = (x @ Wv).reshape(Bsz, Lq, N_HEAD, DV).transpose(0, 2, 1, 3)
    s = np.einsum("bhqd,bhkd->bhqk", Q, K) / np.sqrt(np.float32(DH))
    s = np.where(mask, s, -np.inf)
    s = s - s.max(axis=-1, keepdims=True)
    p = np.exp(s)
    p /= p.sum(axis=-1, keepdims=True)
    ctxv = np.einsum("bhqk,bhkv->bhqv", p, V)
    ctxv = ctxv.transpose(0, 2, 1, 3).reshape(Bsz, Lq, N_HEAD * DV)
    return (ctxv @ Wo + bo).astype(np.float32)


def run_on_hw(in_maps, trace=False):
    from concourse.bass_utils import run_bass_kernel_spmd
    nc = _get_nc()
    return run_bass_kernel_spmd(nc, in_maps, list(range(N_CORES)), trace=trace)


def kernel(x, Wq, Wk, Wv, Wo, bo, mask, _trace=False, _results=None):
    x = np.asarray(x, dtype=np.float32)
    Wq = np.asarray(Wq, dtype=np.float32)
    Wk = np.asarray(Wk, dtype=np.float32)
    Wv = np.asarray(Wv, dtype=np.float32)
    Wo = np.asarray(Wo, dtype=np.float32)
    bo = np.asarray(bo, dtype=np.float32)
    mask_np = np.asarray(mask).reshape(mask.shape[-2], mask.shape[-1])

    causal = bool(np.array_equal(
        mask_np, np.tril(np.ones((L, L), dtype=bool))))
    if not causal or x.shape != (B, L, D):
        return _numpy_fallback(np.asarray(x), Wq, Wk, Wv, Wo, bo,
                               np.asarray(mask))

    res = run_on_hw(make_in_maps(x, Wq, Wk, Wv, Wo), trace=_trace)
    if _results is not None:
        _results.append(res)
    out = np.empty((B, L, D), dtype=np.float32)
    for b in range(B):
        out[b] = (np.asarray(res.results[2 * b]["out"], dtype=np.float32)
                  + np.asarray(res.results[2 * b + 1]["out"], dtype=np.float32)
                  + bo)
    return out


# revision 31
# speedup vs baseline: 1.1070x; 1.0005x over previous
"""Causal multi-head attention Trainium2 kernel (8 NeuronCores).

Problem: B=4, L=2048, D=1024, 16 heads x (dh=64, dv=64), causal mask.
Sharding: data-parallel over batch (4) x tensor-parallel over heads (2 groups
of 8). Core c handles batch c//2, head-group c%2. Each core computes its
partial output projection (ctx_g @ Wo_g); the host sums the two head-group
partials per batch and adds the bias.

v2-v4 (~322us): software-pipelined flash-style attention; S^T tiles = K@Q^T;
exp on ACT (scale=1/8 folded in); causal diagonal via tril tensor_mul +
width-restricted matmuls; ones column of V_aug gives the softmax denominator
in PSUM row 64; bf16 datapath; projections and output projection dribbled
into the attention stream.

v5 (~309us): x arrives pre-transposed from the host; the two heads' S^T
matmuls (contraction 64: head0 on PE rows 0-63/tile T0, head1 rows 64-127/
T8) issue adjacently so the hardware overlaps them on the two 64x128 PE
tiles.

v6: (a) all DRAM operands pre-shuffled on the host into partition-contiguous
layouts (x chunk-major [nch,128,8,512], weights [128,t,o]) so the input DMAs
run at full HBM bandwidth - v5 lost ~15us waiting on 1KB-segment descriptor
patterns at startup. (b) hp3 processes q-chunks descending: the large
l-tiles' output projections dribble into the remaining attention, and the
final drain is only lt 0-3, split into an early v=0..2 partial (pA, dribbled)
plus a 1-matmul finish - the v5 drain ran ~23 matmuls after a >3us PE gap
that re-throttled the HAM clock gate to 1.2 GHz. (c) output stored as bf16
(halves the store bytes; host upcasts and sums the TP partials in f32).
(d) Wq/Wk loads ride the gpsimd queue - their engine-blocking DIRECT2D
triggers were stalling the exp stream on the ACT queue. (e) S quartet issue
order alternates heads (h0r0,h1r0,h0r1,h1r1) for tile overlap.
"""

import numpy as np
from contextlib import ExitStack

import concourse.bass as bass
import concourse.tile as tile
from concourse import bacc, mybir

F32 = mybir.dt.float32
BF16 = mybir.dt.bfloat16
AF = mybir.ActivationFunctionType

B, L, D = 4, 2048, 1024
N_HEAD, DH, DV = 16, 64, 64
N_CORES = 8
HPC = N_HEAD // 2          # heads per core (8)
OC = HPC * DH              # per-core projection width (512)
NHP = HPC // 2             # head-pairs per core (4)


class ProjEmitter:
    """Q^T/K^T projection for one head-pair, emitted in per-(proj,chunk)
    units so the matmuls interleave with attention of the previous pair."""

    def __init__(self, nc, hp, pools, xt, wq, wk, nch, start=0):
        self.nc = nc
        self.xt = xt
        self.start = start
        qkp, wp, self.psP = pools
        self.wq_sb = wp.tile([128, 8, 128], BF16, tag="wq")
        self.wk_sb = wp.tile([128, 8, 128], BF16, tag="wk")
        # gpsimd queue: keeps the engine-blocking DIRECT2D weight triggers
        # off the ACT queue (exps) and the sync queue (normalize bcasts)
        nc.gpsimd.dma_start(out=self.wq_sb, in_=wq[hp])
        nc.gpsimd.dma_start(out=self.wk_sb, in_=wk[hp])
        self.qt = qkp.tile([128, nch * 512], BF16, tag="qt")
        self.kt = qkp.tile([128, nch * 512], BF16, tag="kt")
        self.units = [(w, d, c) for w, d in ((self.wq_sb, self.qt),
                                             (self.wk_sb, self.kt))
                      for c in range(nch)]
        self.i = 0

    def step(self):
        if self.i >= len(self.units):
            return False
        w_sb, dst, c = self.units[self.i]
        self.i += 1
        nc = self.nc
        pp = self.psP.tile([128, 512], F32, tag="pp")
        for d in range(8):
            nc.tensor.matmul(pp, w_sb[:, d, :],
                             self.xt[:, c, d, :],
                             start=(d == 0), stop=(d == 7))
        nc.vector.tensor_copy(dst[:, c * 512:(c + 1) * 512], pp)
        return True

    def drain(self):
        while self.step():
            pass


class OutEmitter:
    """Output projection, dribbled into hp3's attention (hp3 runs its
    q-chunks descending, so l-tiles 12..4 project early); the last chunk's
    tiles 0-3 are split into an early v=0..2 partial plus a single-matmul
    finish so almost nothing runs after the final normalize."""

    def __init__(self, nc, pools, ct, wo_sb, out, drain_lts):
        self.nc = nc
        self.phco, self.psP = pools
        self.ct, self.wo_sb, self.out = ct, wo_sb, out
        self.drain_lts = drain_lts
        self.start = 0
        self.queue = []
        self.ost = {}
        self.pa = {}
        self.dq = 0

    def add_lt(self, lt):
        self.queue.extend([("full", lt, 0), ("full", lt, 1)])

    def add_partial(self, lt):
        self.queue.extend([("pA", lt, 0), ("pA", lt, 1)])

    def add_finish(self, lt):
        self.queue.extend([("fin", lt, 0), ("fin", lt, 1)])

    def step(self):
        if not self.queue:
            return False
        kind, lt, n = self.queue.pop(0)
        nc = self.nc
        pp = self.psP.tile([128, 512], F32, tag="pp", name="ppo")
        if kind == "pA":
            for v in range(3):
                nc.tensor.matmul(pp, self.ct[:, v, lt * 128:(lt + 1) * 128],
                                 self.wo_sb[:, v, n * 512:(n + 1) * 512],
                                 start=(v == 0), stop=(v == 2))
            pa = self.phco.tile([128, 512], F32, tag="pA", bufs=8,
                                name=f"pA{lt}_{n}")
            nc.vector.tensor_copy(pa, pp)
            self.pa[(lt, n)] = pa
            return True
        if n == 0:
            self.ost[lt] = self.phco.tile([128, D], BF16, tag="ost",
                                          name=f"ost{lt}")
        ost = self.ost[lt]
        if kind == "fin":
            nc.tensor.matmul(pp, self.ct[:, 3, lt * 128:(lt + 1) * 128],
                             self.wo_sb[:, 3, n * 512:(n + 1) * 512],
                             start=True, stop=True)
            nc.vector.tensor_add(ost[:, n * 512:(n + 1) * 512], pp,
                                 self.pa.pop((lt, n)))
            # final-tail halves: store immediately, alternating the sync
            # and ACT hw DGE queues (idle by now) so the stores overlap
            eng = nc.sync if self.dq % 2 == 0 else nc.scalar
            self.dq += 1
            eng.dma_start(
                out=self.out[lt * 128:(lt + 1) * 128,
                             n * 512:(n + 1) * 512],
                in_=ost[:, n * 512:(n + 1) * 512])
            if n == 1:
                del self.ost[lt]
            return True
        for v in range(4):
            nc.tensor.matmul(pp, self.ct[:, v, lt * 128:(lt + 1) * 128],
                             self.wo_sb[:, v, n * 512:(n + 1) * 512],
                             start=(v == 0), stop=(v == 3))
        nc.vector.tensor_copy(ost[:, n * 512:(n + 1) * 512], pp)
        if n == 1:
            # alternate the (idle) gpsimd and sync queues so the ~4MiB of
            # output stores drain in parallel instead of piling up
            eng = nc.gpsimd if lt % 2 == 0 else nc.sync
            eng.dma_start(out=self.out[lt * 128:(lt + 1) * 128, :],
                          in_=ost)
            del self.ost[lt]
        return True

    def drain(self):
        while self.step():
            pass


def build_nc(l=L):
    assert l % 512 == 0
    nch = l // 512           # q-chunks
    nlt = l // 128           # l-tiles
    nc = bacc.Bacc("TRN2", target_bir_lowering=False, debug=False,
                   num_devices=N_CORES)

    # all operands pre-shuffled on the host into partition-contiguous
    # DMA layouts (see make_in_maps)
    x = nc.dram_tensor("x", [nch, 128, 8, 512], BF16,
                       kind="ExternalInput").ap()
    wq = nc.dram_tensor("wq", [NHP, 128, 8, 128], BF16,
                        kind="ExternalInput").ap()
    wk = nc.dram_tensor("wk", [NHP, 128, 8, 128], BF16,
                        kind="ExternalInput").ap()
    wv = nc.dram_tensor("wv", [128, 8, OC], BF16, kind="ExternalInput").ap()
    wo = nc.dram_tensor("wo", [128, 4, D], BF16, kind="ExternalInput").ap()
    out = nc.dram_tensor("out", [l, D], BF16, kind="ExternalOutput").ap()

    with tile.TileContext(nc) as tc, ExitStack() as ctx:
        top = ctx.enter_context(tc.tile_pool(name="top", bufs=1))
        xtp = ctx.enter_context(tc.tile_pool(name="xtp", bufs=1))
        qkp = ctx.enter_context(tc.tile_pool(name="qkp", bufs=2))
        wp = ctx.enter_context(tc.tile_pool(name="wp", bufs=2))
        phco = ctx.enter_context(tc.tile_pool(name="phco", bufs=4))

        # V: [128(l), ltile, head, 65] - col 64 is ones (softmax denominator)
        vt = top.tile([128, nlt, HPC, DH + 1], BF16)
        ct = top.tile([128, NHP, l], BF16)        # normalized ctx^T
        tril = top.tile([128, 128], BF16)
        ones = top.tile([128, 1], F32)
        warm = top.tile([128, 1], BF16)
        onesw = top.tile([1, DV], BF16)           # rank-1 bcast weights
        xt = xtp.tile([128, nch, 8, 512], BF16)   # x^T, chunk-major

        wones = top.tile([128, 1], BF16)
        wbuf = top.tile([128, 256], BF16)

        nc.vector.memset(ones, 1.0)
        nc.vector.memset(onesw, 1.0)
        nc.vector.memset(wones, 1.0)
        nc.vector.memset(wbuf, 0.0)
        # warm-up exp: loads the ACT function table during the DMA-bound
        # startup instead of stalling the first attention group
        nc.scalar.activation(warm, ones, AF.Exp, scale=0.125)
        nc.vector.tensor_copy(
            vt[:, :, :, DV:DV + 1].rearrange("p t h c -> p (t h) c"),
            ones.broadcast_to((128, nlt * HPC, 1)))
        # causal keep-mask for S^T diag blocks: tril[k, q] = 1.0 iff q >= k
        nc.gpsimd.memset(tril, 0.0)
        nc.gpsimd.affine_select(
            out=tril, in_=tril, compare_op=mybir.AluOpType.is_gt,
            fill=1.0, base=0, pattern=[[-1, 128]], channel_multiplier=1)

        # ---------------- Prologue: xT DMA + V + QK(hp=0) -----------------
        with tc.tile_pool(name="wvp", bufs=1) as wvp, \
             tc.tile_pool(name="psPro", bufs=3, space="PSUM") as psPro:
            # DMA staging: the device DMA pipe is shared (8 cores pull
            # ~50MiB of inputs concurrently at kernel start, ~150-250 GB/s
            # effective per core), so ARRIVAL ORDER is everything. Transfers
            # on one queue run FIFO - putting all x chunks on sync in need
            # order stages them automatically; wv/wo ride scalar, Wq/Wk
            # gpsimd, so at most ~3 transfers share the pipe at once.
            wv_sb = wvp.tile([128, 8, OC], BF16)
            nc.scalar.dma_start(out=wv_sb[:, 0:4, :], in_=wv[:, 0:4, :])
            nc.sync.dma_start(out=xt[:, 0, :, 0:256], in_=x[0, :, :, 0:256])
            nc.scalar.dma_start(out=wv_sb[:, 4:8, :], in_=wv[:, 4:8, :])
            nc.sync.dma_start(out=xt[:, 0, :, 256:512],
                              in_=x[0, :, :, 256:512])
            for c in range(1, nch):
                nc.sync.dma_start(out=xt[:, c], in_=x[c])
            # ~24 rank-1 warm-up matmuls fill the DMA wait with PE activity
            # so the HAM clock gate reaches 8/8 before the first real matmul
            for w in range(24):
                pw = psPro.tile([1, 256], F32, tag="pwarm", bufs=2)
                nc.tensor.matmul(pw, wones, wbuf, start=True, stop=True)
            for c in range(nch):
                if c == 1:
                    # hp0's Wq/Wk ride the (idle) gpsimd queue
                    em = ProjEmitter(nc, 0, (qkp, wp, psPro), xt, wq, wk, nch)
                # V for this l-chunk
                for m in range(4):
                    pp = psPro.tile([128, 512], F32, tag="pp")
                    for d in range(8):
                        nc.tensor.matmul(
                            pp, xt[:, c, d, m * 128:(m + 1) * 128],
                            wv_sb[:, d, :], start=(d == 0), stop=(d == 7))
                        if c == 0 and m == 0 and d == 3:
                            # chunk 0 stalls here on the wv/x second-half
                            # DMAs; keep the HAM clock gate warm meanwhile
                            for w in range(12):
                                pw = psPro.tile([1, 256], F32, tag="pwarm",
                                                bufs=2)
                                nc.tensor.matmul(pw, wones, wbuf,
                                                 start=True, stop=True)
                    nc.vector.tensor_copy(
                        vt[:, c * 4 + m, :, 0:DV],
                        pp.rearrange("p (h v) -> p h v", h=HPC))
            em.drain()

        # Prefetch Wo now: the scalar DMA queue drains during early hp0
        # attention, long before the output projection needs it.
        phc = ctx.enter_context(tc.tile_pool(name="phc", bufs=1))
        wo_sb = phc.tile([128, 4, D], BF16)
        nc.scalar.dma_start(out=wo_sb, in_=wo)

        # ---------------- Main: attention + next-pair projections ---------
        with tc.tile_pool(name="phb", bufs=2) as phb, \
             tc.tile_pool(name="psS", bufs=2, space="PSUM") as psS, \
             tc.tile_pool(name="psPd", bufs=2, space="PSUM") as psPd, \
             tc.tile_pool(name="psC", bufs=2, space="PSUM") as psC:
            n_groups_hp = 2 * nch * (nch + 1)
            for hp in range(NHP):
                qt, kt = em.qt, em.kt
                if hp + 1 < NHP:
                    # hp0 runs j descending, so its shallow (bubble-prone)
                    # chunks come last: dribble the projection units there.
                    # Ascending hps dribble early (after the Wq/Wk DMA
                    # lands) into their shallow first chunks.
                    em = ProjEmitter(nc, hp + 1, (qkp, wp, psPd), xt, wq, wk,
                                     nch, start=24 if hp == 0 else 4)
                    cadence = 1
                else:
                    em = OutEmitter(nc, (phco, psPd), ct, wo_sb, out,
                                    drain_lts=range(0, 4))
                    # lt 0-3's head-pair 0-2 ctx has been ready since hp2:
                    # dribble their v=0..2 partials during hp3's first chunk
                    for lt in range(4):
                        em.add_partial(lt)
                    cadence = 1
                gcount = 0

                # hp0 and hp3 run j descending: hp0 so j=3's 32 S matmuls
                # hide the cold-ACT exp latency at the prologue boundary,
                # hp3 so the big l-tiles' out-projection dribbles early and
                # only lt 0-3 remain for the drain.
                desc = hp == 0 or hp == NHP - 1
                jorder = range(nch - 1, -1, -1) if desc else range(nch)
                last_j = 0 if desc else nch - 1
                for j in jorder:
                    n_g = 2 * (j + 1)
                    pctxs = {}
                    prev = None
                    final_chunk = hp == NHP - 1 and j == last_j
                    rss, ctus, invs, invbs = {}, {}, {}, {}
                    for h in range(2):
                        pctxs[h] = psC.tile([DV + 1, 512], F32,
                                            tag="pctx", name=f"pctx{h}")
                    for g in range(n_g + 1):
                        # k-tile-major psc tiles [128, head, q]: the two
                        # heads' S^T matmuls per k-tile issue back-to-back
                        # (head0 on PE rows 0-63/tile T0, head1 on rows
                        # 64-127/T8) and overlap on the two 64x128 PE
                        # tiles; each k-tile's exp covers exactly its
                        # causal width (no garbage columns), and the psc
                        # ring gates the next group's first S pair on the
                        # EARLY exp of this group.
                        pexps = {}
                        if g < n_g:
                            for r2 in range(2):
                                kt_i = 2 * g + r2
                                c0 = max(0, kt_i - 4 * j) * 128
                                psc = psS.tile([128, 2, 512], F32,
                                               tag="psc", name=f"psc{r2}")
                                for h in range(2):
                                    po = 64 * h
                                    nc.tensor.matmul(
                                        psc[:, h, c0:512],
                                        kt[po:po + DH,
                                           kt_i * 128:(kt_i + 1) * 128],
                                        qt[po:po + DH,
                                           j * 512 + c0:(j + 1) * 512],
                                        start=True, stop=True)
                                pexp = phb.tile([128, 2, 512], BF16,
                                                tag="pexp", bufs=10,
                                                name=f"pexp{r2}")
                                nc.scalar.activation(pexp[:, :, c0:512],
                                                     psc[:, :, c0:512],
                                                     AF.Exp, scale=0.125)
                                pexps[r2] = (pexp, c0)
                        if prev is not None:
                            pg, ppexps = prev
                            # causal keep-mask on diagonal k-tiles, both
                            # heads in one multiply
                            for r2 in range(2):
                                kt_i = 2 * pg + r2
                                if kt_i - 4 * j >= 0:
                                    c0 = (kt_i - 4 * j) * 128
                                    pexp = ppexps[r2][0]
                                    nc.vector.tensor_mul(
                                        pexp[:, :, c0:c0 + 128],
                                        pexp[:, :, c0:c0 + 128],
                                        tril.rearrange("p (o q) -> p o q",
                                                       o=1)
                                        .broadcast_to((128, 2, 128)))
                            for h in range(2):
                                H = 2 * hp + h
                                for r2 in range(2):
                                    kt_i = 2 * pg + r2
                                    pexp, c0 = ppexps[r2]
                                    nc.tensor.matmul(
                                        pctxs[h][:, c0:512],
                                        vt[:, kt_i, H, :],
                                        pexp[:, h, c0:512],
                                        start=(kt_i == 0),
                                        stop=(kt_i == 4 * j + 3))
                                if final_chunk and g == n_g:
                                    # issue this head's normalize DVE chain
                                    # right behind its last PV so the
                                    # rank-1 broadcast below never waits
                                    rss[h] = phb.tile([1, 512], F32,
                                                      tag="rs",
                                                      name=f"rs{h}")
                                    nc.vector.tensor_copy(
                                        rss[h], pctxs[h][DV:DV + 1, :])
                                    ctus[h] = phb.tile([64, 512], BF16,
                                                       tag="ctu",
                                                       name=f"ctu{h}")
                                    nc.vector.tensor_copy(
                                        ctus[h], pctxs[h][0:DV, :])
                                    invs[h] = phb.tile([1, 512], F32,
                                                       tag="inv",
                                                       name=f"inv{h}")
                                    nc.vector.reciprocal_approx_fast(
                                        out=invs[h], in_=rss[h])
                                    invbs[h] = phb.tile([1, 512], BF16,
                                                        tag="invb",
                                                        name=f"invb{h}")
                                    nc.vector.tensor_copy(invbs[h],
                                                          invs[h])
                                gcount += 1
                                if (em and cadence and gcount > em.start
                                        and gcount % cadence == 0):
                                    em.step()
                        prev = (g, pexps) if g < n_g else None
                    if final_chunk:
                        # flush remaining independent out-proj units BEFORE
                        # the final normalize: the normalize-dependent rank-1
                        # broadcasts otherwise block them at the head of the
                        # strict-FIFO PE queue (and the idle re-throttles the
                        # HAM clock gate, running the whole drain at 1.2GHz).
                        # The DVE chains were issued inside the PV loop and
                        # run concurrently with these matmuls.
                        em.drain()
                        for h in range(2):
                            po = 64 * h
                            bcp = psPd.tile([128, 512], F32, tag="pp",
                                            name=f"bcp{h}")
                            nc.tensor.matmul(bcp[0:DV, :], onesw,
                                             invbs[h], start=True, stop=True)
                            nc.vector.tensor_mul(
                                ct[po:po + DV, hp, j * 512:(j + 1) * 512],
                                ctus[h], bcp[0:DV, :])
                    else:
                        for h in range(2):
                            po = 64 * h
                            # free the pctx PSUM bank after one DVE copy +
                            # a PSUM-direct recip (the next j's first PV
                            # waits on it); broadcast/scale dangle off the
                            # critical path.
                            rs = phb.tile([1, 512], F32, tag="rs",
                                          name="rs")
                            nc.vector.tensor_copy(rs,
                                                  pctxs[h][DV:DV + 1, :])
                            ctu = phb.tile([64, 512], BF16, tag="ctu",
                                           name=f"ctu{h}")
                            nc.vector.tensor_copy(ctu, pctxs[h][0:DV, :])
                            inv = phb.tile([1, 512], F32, tag="inv",
                                           name="inv")
                            nc.vector.reciprocal_approx_fast(
                                out=inv, in_=rs)
                            bc = phb.tile([64, 512], F32, tag="bc",
                                          name="bc")
                            nc.gpsimd.partition_broadcast(out_ap=bc,
                                                          in_ap=inv)
                            nc.vector.tensor_mul(
                                ct[po:po + DV, hp, j * 512:(j + 1) * 512],
                                ctu, bc)
                    if hp == NHP - 1:
                        if j == last_j:
                            em.add_finish(0)
                            em.add_finish(1)
                            em.add_finish(2)
                            em.add_finish(3)
                        else:
                            for lt in range(4 * j, 4 * j + 4):
                                em.add_lt(lt)
                em.drain()

    nc.compile()
    return nc


def _bf16(a):
    import ml_dtypes
    return np.ascontiguousarray(a).astype(ml_dtypes.bfloat16)


def make_in_maps(x, Wq, Wk, Wv, Wo):
    nch = L // 512
    in_maps = []
    for c in range(N_CORES):
        b, g = c // 2, c % 2
        # x^T chunk-major: [c, p, t, lc] = x[b][c*512+lc, t*128+p]
        xdev = x[b].reshape(nch, 512, 8, 128).transpose(0, 3, 2, 1)
        wqg = Wq[:, g * OC:(g + 1) * OC]
        wkg = Wk[:, g * OC:(g + 1) * OC]
        # per-head-pair [hp, p, t, o] = W[t*128+p, hp*128+o]
        wqd = wqg.reshape(8, 128, NHP, 128).transpose(2, 1, 0, 3)
        wkd = wkg.reshape(8, 128, NHP, 128).transpose(2, 1, 0, 3)
        # [p, t, o] = W[t*128+p, o]
        wvd = Wv[:, g * OC:(g + 1) * OC].reshape(8, 128, OC).transpose(1, 0, 2)
        # [p, v, o] = Wo[v*128+p, o]
        wod = Wo[g * OC:(g + 1) * OC, :].reshape(4, 128, D).transpose(1, 0, 2)
        in_maps.append({
            "x": _bf16(xdev),
            "wq": _bf16(wqd),
            "wk": _bf16(wkd),
            "wv": _bf16(wvd),
            "wo": _bf16(wod),
        })
    return in_maps


_NC_CACHE = {}


def _get_nc():
    if "nc" not in _NC_CACHE:
        _NC_CACHE["nc"] = build_nc()
    return _NC_CACHE["nc"]


def _numpy_fallback(x, Wq, Wk, Wv, Wo, bo, mask):
    Bsz, Lq, _ = x.shape
    Q = (x @ Wq).reshape(Bsz, Lq, N_HEAD, DH).transpose(0, 2, 1, 3)
    K = (x @ Wk).reshape(Bsz, Lq, N_HEAD, DH).transpose(0, 2, 1, 3)
    V = (x @ Wv).reshape(Bsz, Lq, N_HEAD, DV).transpose(0, 2, 1, 3)
    s = np.einsum("bhqd,bhkd->bhqk", Q, K) / np.sqrt(np.float32(DH))
    s = np.where(mask, s, -np.inf)
    s = s - s.max(axis=-1, keepdims=True)
    p = np.exp(s)
    p /= p.sum(axis=-1, keepdims=True)
    ctxv = np.einsum("bhqk,bhkv->bhqv", p, V)
    ctxv = ctxv.transpose(0, 2, 1, 3).reshape(Bsz, Lq, N_HEAD * DV)
    return (ctxv @ Wo + bo).astype(np.float32)


def run_on_hw(in_maps, trace=False):
    from concourse.bass_utils import run_bass_kernel_spmd
    nc = _get_nc()
    return run_bass_kernel_spmd(nc, in_maps, list(range(N_CORES)), trace=trace)


def kernel(x, Wq, Wk, Wv, Wo, bo, mask, _trace=False, _results=None):
    x = np.asarray(x, dtype=np.float32)
    Wq = np.asarray(Wq, dtype=np.float32)
    Wk = np.asarray(Wk, dtype=np.float32)
    Wv = np.asarray(Wv, dtype=np.float32)
    Wo = np.asarray(Wo, dtype=np.float32)
    bo = np.asarray(bo, dtype=np.float32)
    mask_np = np.asarray(mask).reshape(mask.shape[-2], mask.shape[-1])

    causal = bool(np.array_equal(
        mask_np, np.tril(np.ones((L, L), dtype=bool))))
    if not causal or x.shape != (B, L, D):
        return _numpy_fallback(np.asarray(x), Wq, Wk, Wv, Wo, bo,
                               np.asarray(mask))

    res = run_on_hw(make_in_maps(x, Wq, Wk, Wv, Wo), trace=_trace)
    if _results is not None:
        _results.append(res)
    out = np.empty((B, L, D), dtype=np.float32)
    for b in range(B):
        out[b] = (np.asarray(res.results[2 * b]["out"], dtype=np.float32)
                  + np.asarray(res.results[2 * b + 1]["out"], dtype=np.float32)
                  + bo)
    return out


# revision 32
# speedup vs baseline: 1.1285x; 1.0195x over previous
"""Causal multi-head attention Trainium2 kernel (8 NeuronCores).

Problem: B=4, L=2048, D=1024, 16 heads x (dh=64, dv=64), causal mask.
Sharding: data-parallel over batch (4) x tensor-parallel over heads (2 groups
of 8). Core c handles batch c//2, head-group c%2. Each core computes its
partial output projection (ctx_g @ Wo_g); the host sums the two head-group
partials per batch and adds the bias.

v2-v4 (~322us): software-pipelined flash-style attention; S^T tiles = K@Q^T;
exp on ACT (scale=1/8 folded in); causal diagonal via tril tensor_mul +
width-restricted matmuls; ones column of V_aug gives the softmax denominator
in PSUM row 64; bf16 datapath; projections and output projection dribbled
into the attention stream.

v5 (~309us): x arrives pre-transposed from the host; the two heads' S^T
matmuls (contraction 64: head0 on PE rows 0-63/tile T0, head1 rows 64-127/
T8) issue adjacently so the hardware overlaps them on the two 64x128 PE
tiles.

v6-v10: (a) all DRAM operands pre-shuffled on the host into
partition-contiguous layouts; x chunks ride ONE queue FIFO so arrival order
matches need order (the DMA pipe round-robins in-flight transfers across 8
cores at ~150-250 GB/s/core - issuing everything upfront made the
first-needed tensor finish last). (b) hp3 processes q-chunks descending: the
big l-tiles' output projections dribble into the remaining attention and
only lt 0-3 remain for the drain, split into an early v=0..2 partial (pA)
plus a 1-matmul finish. (c) output stored as bf16, stores spread across
gpsimd/sync/scalar queues. (d) Wq/Wk loads on the gpsimd queue (their
engine-blocking DIRECT2D triggers were stalling the exp stream on the ACT
queue). (e) ~24 rank-1 warm-up matmuls during the DMA wait so the HAM clock
gate is at 8/8 before the first real matmul.

v11-v12 (295us, from 320us baseline): k-tile-major psc tiles [128, head, q].
The two heads' S^T matmuls per k-tile issue back-to-back with nothing
between them - head0 runs on PE rows 0-63 (64x128 tile T0), head1 on rows
64-127 (T8), and the two tiles execute CONCURRENTLY (~2x on the S stream;
tile_position auto-derives from the qt/kt base partitions). Each k-tile's
exp covers exactly its causal width (the old head-major grouping exp'd 8192
garbage columns), the diagonal tril mask multiplies both heads in one DVE
op, and the psc ring gates the next group's first S pair on the EARLY exp
of the previous group instead of the late one. The final chunk's normalize
DVE chains issue directly behind each head's last PV so the closing rank-1
broadcast matmuls never idle the PE.
"""

import numpy as np
from contextlib import ExitStack

import concourse.bass as bass
import concourse.tile as tile
from concourse import bacc, mybir

F32 = mybir.dt.float32
BF16 = mybir.dt.bfloat16
AF = mybir.ActivationFunctionType

B, L, D = 4, 2048, 1024
N_HEAD, DH, DV = 16, 64, 64
N_CORES = 8
HPC = N_HEAD // 2          # heads per core (8)
OC = HPC * DH              # per-core projection width (512)
NHP = HPC // 2             # head-pairs per core (4)


class ProjEmitter:
    """Q^T/K^T projection for one head-pair, emitted in per-(proj,chunk)
    units so the matmuls interleave with attention of the previous pair."""

    def __init__(self, nc, hp, pools, xt, wq, wk, nch, start=0):
        self.nc = nc
        self.xt = xt
        self.start = start
        qkp, wp, self.psP = pools
        self.wq_sb = wp.tile([128, 8, 128], BF16, tag="wq")
        self.wk_sb = wp.tile([128, 8, 128], BF16, tag="wk")
        # gpsimd queue: keeps the engine-blocking DIRECT2D weight triggers
        # off the ACT queue (exps) and the sync queue (normalize bcasts)
        nc.gpsimd.dma_start(out=self.wq_sb, in_=wq[hp])
        nc.gpsimd.dma_start(out=self.wk_sb, in_=wk[hp])
        self.qt = qkp.tile([128, nch * 512], BF16, tag="qt")
        self.kt = qkp.tile([128, nch * 512], BF16, tag="kt")
        self.units = [(w, d, c) for w, d in ((self.wq_sb, self.qt),
                                             (self.wk_sb, self.kt))
                      for c in range(nch)]
        self.i = 0

    def step(self):
        if self.i >= len(self.units):
            return False
        w_sb, dst, c = self.units[self.i]
        self.i += 1
        nc = self.nc
        pp = self.psP.tile([128, 512], F32, tag="pp")
        for d in range(8):
            nc.tensor.matmul(pp, w_sb[:, d, :],
                             self.xt[:, c, d, :],
                             start=(d == 0), stop=(d == 7))
        nc.vector.tensor_copy(dst[:, c * 512:(c + 1) * 512], pp)
        return True

    def drain(self):
        while self.step():
            pass


class OutEmitter:
    """Output projection, dribbled into hp3's attention (hp3 runs its
    q-chunks descending, so l-tiles 12..4 project early); the last chunk's
    tiles 0-3 are split into an early v=0..2 partial plus a single-matmul
    finish so almost nothing runs after the final normalize."""

    def __init__(self, nc, pools, ct, wo_sb, out, drain_lts):
        self.nc = nc
        self.phco, self.psP = pools
        self.ct, self.wo_sb, self.out = ct, wo_sb, out
        self.drain_lts = drain_lts
        self.start = 0
        self.queue = []
        self.ost = {}
        self.pa = {}
        self.dq = 0

    def add_lt(self, lt):
        self.queue.extend([("full", lt, 0), ("full", lt, 1)])

    def add_partial(self, lt):
        self.queue.extend([("pA", lt, 0), ("pA", lt, 1)])

    def add_finish(self, lt):
        self.queue.extend([("fin", lt, 0), ("fin", lt, 1)])

    def step(self):
        if not self.queue:
            return False
        kind, lt, n = self.queue.pop(0)
        nc = self.nc
        pp = self.psP.tile([128, 512], F32, tag="pp", name="ppo")
        if kind == "pA":
            for v in range(3):
                nc.tensor.matmul(pp, self.ct[:, v, lt * 128:(lt + 1) * 128],
                                 self.wo_sb[:, v, n * 512:(n + 1) * 512],
                                 start=(v == 0), stop=(v == 2))
            pa = self.phco.tile([128, 512], F32, tag="pA", bufs=8,
                                name=f"pA{lt}_{n}")
            nc.vector.tensor_copy(pa, pp)
            self.pa[(lt, n)] = pa
            return True
        if n == 0:
            self.ost[lt] = self.phco.tile([128, D], BF16, tag="ost",
                                          name=f"ost{lt}")
        ost = self.ost[lt]
        if kind == "fin":
            nc.tensor.matmul(pp, self.ct[:, 3, lt * 128:(lt + 1) * 128],
                             self.wo_sb[:, 3, n * 512:(n + 1) * 512],
                             start=True, stop=True)
            nc.vector.tensor_add(ost[:, n * 512:(n + 1) * 512], pp,
                                 self.pa.pop((lt, n)))
            # final-tail halves: store immediately, alternating the sync
            # and ACT hw DGE queues (idle by now) so the stores overlap
            eng = nc.sync if self.dq % 2 == 0 else nc.scalar
            self.dq += 1
            eng.dma_start(
                out=self.out[lt * 128:(lt + 1) * 128,
                             n * 512:(n + 1) * 512],
                in_=ost[:, n * 512:(n + 1) * 512])
            if n == 1:
                del self.ost[lt]
            return True
        for v in range(4):
            nc.tensor.matmul(pp, self.ct[:, v, lt * 128:(lt + 1) * 128],
                             self.wo_sb[:, v, n * 512:(n + 1) * 512],
                             start=(v == 0), stop=(v == 3))
        nc.vector.tensor_copy(ost[:, n * 512:(n + 1) * 512], pp)
        if n == 1:
            # alternate the (idle) gpsimd and sync queues so the ~4MiB of
            # output stores drain in parallel instead of piling up
            eng = nc.gpsimd if lt % 2 == 0 else nc.sync
            eng.dma_start(out=self.out[lt * 128:(lt + 1) * 128, :],
                          in_=ost)
            del self.ost[lt]
        return True

    def drain(self):
        while self.step():
            pass


def build_nc(l=L):
    assert l % 512 == 0
    nch = l // 512           # q-chunks
    nlt = l // 128           # l-tiles
    nc = bacc.Bacc("TRN2", target_bir_lowering=False, debug=False,
                   num_devices=N_CORES)

    # all operands pre-shuffled on the host into partition-contiguous
    # DMA layouts (see make_in_maps)
    x = nc.dram_tensor("x", [nch, 128, 8, 512], BF16,
                       kind="ExternalInput").ap()
    wq = nc.dram_tensor("wq", [NHP, 128, 8, 128], BF16,
                        kind="ExternalInput").ap()
    wk = nc.dram_tensor("wk", [NHP, 128, 8, 128], BF16,
                        kind="ExternalInput").ap()
    wv = nc.dram_tensor("wv", [128, 8, OC], BF16, kind="ExternalInput").ap()
    wo = nc.dram_tensor("wo", [128, 4, D], BF16, kind="ExternalInput").ap()
    out = nc.dram_tensor("out", [l, D], BF16, kind="ExternalOutput").ap()

    with tile.TileContext(nc) as tc, ExitStack() as ctx:
        top = ctx.enter_context(tc.tile_pool(name="top", bufs=1))
        xtp = ctx.enter_context(tc.tile_pool(name="xtp", bufs=1))
        qkp = ctx.enter_context(tc.tile_pool(name="qkp", bufs=2))
        wp = ctx.enter_context(tc.tile_pool(name="wp", bufs=2))
        phco = ctx.enter_context(tc.tile_pool(name="phco", bufs=4))

        # V: [128(l), ltile, head, 65] - col 64 is ones (softmax denominator)
        vt = top.tile([128, nlt, HPC, DH + 1], BF16)
        ct = top.tile([128, NHP, l], BF16)        # normalized ctx^T
        tril = top.tile([128, 128], BF16)
        ones = top.tile([128, 1], F32)
        warm = top.tile([128, 1], BF16)
        onesw = top.tile([1, DV], BF16)           # rank-1 bcast weights
        xt = xtp.tile([128, nch, 8, 512], BF16)   # x^T, chunk-major

        wones = top.tile([128, 1], BF16)
        wbuf = top.tile([128, 256], BF16)

        nc.vector.memset(ones, 1.0)
        nc.vector.memset(onesw, 1.0)
        nc.vector.memset(wones, 1.0)
        nc.vector.memset(wbuf, 0.0)
        # warm-up exp: loads the ACT function table during the DMA-bound
        # startup instead of stalling the first attention group
        nc.scalar.activation(warm, ones, AF.Exp, scale=0.125)
        nc.vector.tensor_copy(
            vt[:, :, :, DV:DV + 1].rearrange("p t h c -> p (t h) c"),
            ones.broadcast_to((128, nlt * HPC, 1)))
        # causal keep-mask for S^T diag blocks: tril[k, q] = 1.0 iff q >= k
        nc.gpsimd.memset(tril, 0.0)
        nc.gpsimd.affine_select(
            out=tril, in_=tril, compare_op=mybir.AluOpType.is_gt,
            fill=1.0, base=0, pattern=[[-1, 128]], channel_multiplier=1)

        # ---------------- Prologue: xT DMA + V + QK(hp=0) -----------------
        with tc.tile_pool(name="wvp", bufs=1) as wvp, \
             tc.tile_pool(name="psPro", bufs=3, space="PSUM") as psPro:
            # DMA staging: the device DMA pipe is shared (8 cores pull
            # ~50MiB of inputs concurrently at kernel start, ~150-250 GB/s
            # effective per core), so ARRIVAL ORDER is everything. Transfers
            # on one queue run FIFO - putting all x chunks on sync in need
            # order stages them automatically; wv/wo ride scalar, Wq/Wk
            # gpsimd, so at most ~3 transfers share the pipe at once.
            wv_sb = wvp.tile([128, 8, OC], BF16)
            nc.scalar.dma_start(out=wv_sb[:, 0:4, :], in_=wv[:, 0:4, :])
            nc.sync.dma_start(out=xt[:, 0, :, 0:256], in_=x[0, :, :, 0:256])
            nc.scalar.dma_start(out=wv_sb[:, 4:8, :], in_=wv[:, 4:8, :])
            nc.sync.dma_start(out=xt[:, 0, :, 256:512],
                              in_=x[0, :, :, 256:512])
            for c in range(1, nch):
                nc.sync.dma_start(out=xt[:, c], in_=x[c])
            # ~24 rank-1 warm-up matmuls fill the DMA wait with PE activity
            # so the HAM clock gate reaches 8/8 before the first real matmul
            for w in range(24):
                pw = psPro.tile([1, 256], F32, tag="pwarm", bufs=2)
                nc.tensor.matmul(pw, wones, wbuf, start=True, stop=True)
            for c in range(nch):
                if c == 1:
                    # hp0's Wq/Wk ride the (idle) gpsimd queue
                    em = ProjEmitter(nc, 0, (qkp, wp, psPro), xt, wq, wk, nch)
                # V for this l-chunk
                for m in range(4):
                    pp = psPro.tile([128, 512], F32, tag="pp")
                    for d in range(8):
                        nc.tensor.matmul(
                            pp, xt[:, c, d, m * 128:(m + 1) * 128],
                            wv_sb[:, d, :], start=(d == 0), stop=(d == 7))
                        if c == 0 and m == 0 and d == 3:
                            # chunk 0 stalls here on the wv/x second-half
                            # DMAs; keep the HAM clock gate warm meanwhile
                            for w in range(12):
                                pw = psPro.tile([1, 256], F32, tag="pwarm",
                                                bufs=2)
                                nc.tensor.matmul(pw, wones, wbuf,
                                                 start=True, stop=True)
                    nc.vector.tensor_copy(
                        vt[:, c * 4 + m, :, 0:DV],
                        pp.rearrange("p (h v) -> p h v", h=HPC))
            em.drain()

        # Prefetch Wo now: the scalar DMA queue drains during early hp0
        # attention, long before the output projection needs it.
        phc = ctx.enter_context(tc.tile_pool(name="phc", bufs=1))
        wo_sb = phc.tile([128, 4, D], BF16)
        nc.scalar.dma_start(out=wo_sb, in_=wo)

        # ---------------- Main: attention + next-pair projections ---------
        with tc.tile_pool(name="phb", bufs=2) as phb, \
             tc.tile_pool(name="psS", bufs=2, space="PSUM") as psS, \
             tc.tile_pool(name="psPd", bufs=2, space="PSUM") as psPd, \
             tc.tile_pool(name="psC", bufs=2, space="PSUM") as psC:
            n_groups_hp = 2 * nch * (nch + 1)
            for hp in range(NHP):
                qt, kt = em.qt, em.kt
                if hp + 1 < NHP:
                    # hp0 runs j descending, so its shallow (bubble-prone)
                    # chunks come last: dribble the projection units there.
                    # Ascending hps dribble early (after the Wq/Wk DMA
                    # lands) into their shallow first chunks.
                    em = ProjEmitter(nc, hp + 1, (qkp, wp, psPd), xt, wq, wk,
                                     nch, start=24 if hp == 0 else 4)
                    cadence = 1
                else:
                    em = OutEmitter(nc, (phco, psPd), ct, wo_sb, out,
                                    drain_lts=range(0, 4))
                    # lt 0-3's head-pair 0-2 ctx has been ready since hp2:
                    # dribble their v=0..2 partials during hp3's first chunk
                    for lt in range(4):
                        em.add_partial(lt)
                    cadence = 1
                gcount = 0

                # hp0 and hp3 run j descending: hp0 so j=3's 32 S matmuls
                # hide the cold-ACT exp latency at the prologue boundary,
                # hp3 so the big l-tiles' out-projection dribbles early and
                # only lt 0-3 remain for the drain.
                desc = hp == 0 or hp == NHP - 1
                jorder = range(nch - 1, -1, -1) if desc else range(nch)
                last_j = 0 if desc else nch - 1
                for j in jorder:
                    n_g = 2 * (j + 1)
                    pctxs = {}
                    prev = None
                    final_chunk = hp == NHP - 1 and j == last_j
                    rss, ctus, invs, invbs = {}, {}, {}, {}
                    for h in range(2):
                        pctxs[h] = psC.tile([DV + 1, 512], F32,
                                            tag="pctx", name=f"pctx{h}")
                    for g in range(n_g + 1):
                        # k-tile-major psc tiles [128, head, q]: the two
                        # heads' S^T matmuls per k-tile issue back-to-back
                        # (head0 on PE rows 0-63/tile T0, head1 on rows
                        # 64-127/T8) and overlap on the two 64x128 PE
                        # tiles; each k-tile's exp covers exactly its
                        # causal width (no garbage columns), and the psc
                        # ring gates the next group's first S pair on the
                        # EARLY exp of this group.
                        pexps = {}
                        if g < n_g:
                            for r2 in range(2):
                                kt_i = 2 * g + r2
                                c0 = max(0, kt_i - 4 * j) * 128
                                psc = psS.tile([128, 2, 512], F32,
                                               tag="psc", name=f"psc{r2}")
                                for h in range(2):
                                    po = 64 * h
                                    nc.tensor.matmul(
                                        psc[:, h, c0:512],
                                        kt[po:po + DH,
                                           kt_i * 128:(kt_i + 1) * 128],
                                        qt[po:po + DH,
                                           j * 512 + c0:(j + 1) * 512],
                                        start=True, stop=True)
                                pexp = phb.tile([128, 2, 512], BF16,
                                                tag="pexp", bufs=10,
                                                name=f"pexp{r2}")
                                nc.scalar.activation(pexp[:, :, c0:512],
                                                     psc[:, :, c0:512],
                                                     AF.Exp, scale=0.125)
                                pexps[r2] = (pexp, c0)
                        if prev is not None:
                            pg, ppexps = prev
                            # causal keep-mask on diagonal k-tiles, both
                            # heads in one multiply
                            for r2 in range(2):
                                kt_i = 2 * pg + r2
                                if kt_i - 4 * j >= 0:
                                    c0 = (kt_i - 4 * j) * 128
                                    pexp = ppexps[r2][0]
                                    nc.vector.tensor_mul(
                                        pexp[:, :, c0:c0 + 128],
                                        pexp[:, :, c0:c0 + 128],
                                        tril.rearrange("p (o q) -> p o q",
                                                       o=1)
                                        .broadcast_to((128, 2, 128)))
                            for h in range(2):
                                H = 2 * hp + h
                                for r2 in range(2):
                                    kt_i = 2 * pg + r2
                                    pexp, c0 = ppexps[r2]
                                    nc.tensor.matmul(
                                        pctxs[h][:, c0:512],
                                        vt[:, kt_i, H, :],
                                        pexp[:, h, c0:512],
                                        start=(kt_i == 0),
                                        stop=(kt_i == 4 * j + 3))
                                if final_chunk and g == n_g:
                                    # issue this head's normalize DVE chain
                                    # right behind its last PV so the
                                    # rank-1 broadcast below never waits
                                    rss[h] = phb.tile([1, 512], F32,
                                                      tag="rs",
                                                      name=f"rs{h}")
                                    nc.vector.tensor_copy(
                                        rss[h], pctxs[h][DV:DV + 1, :])
                                    ctus[h] = phb.tile([64, 512], BF16,
                                                       tag="ctu",
                                                       name=f"ctu{h}")
                                    nc.vector.tensor_copy(
                                        ctus[h], pctxs[h][0:DV, :])
                                    invs[h] = phb.tile([1, 512], F32,
                                                       tag="inv",
                                                       name=f"inv{h}")
                                    nc.vector.reciprocal_approx_fast(
                                        out=invs[h], in_=rss[h])
                                    invbs[h] = phb.tile([1, 512], BF16,
                                                        tag="invb",
                                                        name=f"invb{h}")
                                    nc.vector.tensor_copy(invbs[h],
                                                          invs[h])
                                gcount += 1
                                if (em and cadence and gcount > em.start
                                        and gcount % cadence == 0):
                                    em.step()
                        prev = (g, pexps) if g < n_g else None
                    if final_chunk:
                        # flush remaining independent out-proj units BEFORE
                        # the final normalize: the normalize-dependent rank-1
                        # broadcasts otherwise block them at the head of the
                        # strict-FIFO PE queue (and the idle re-throttles the
                        # HAM clock gate, running the whole drain at 1.2GHz).
                        # The DVE chains were issued inside the PV loop and
                        # run concurrently with these matmuls.
                        em.drain()
                        for h in range(2):
                            po = 64 * h
                            bcp = psPd.tile([128, 512], F32, tag="pp",
                                            name=f"bcp{h}")
                            nc.tensor.matmul(bcp[0:DV, :], onesw,
                                             invbs[h], start=True, stop=True)
                            nc.vector.tensor_mul(
                                ct[po:po + DV, hp, j * 512:(j + 1) * 512],
                                ctus[h], bcp[0:DV, :])
                    else:
                        for h in range(2):
                            po = 64 * h
                            # free the pctx PSUM bank after one DVE copy +
                            # a PSUM-direct recip (the next j's first PV
                            # waits on it); broadcast/scale dangle off the
                            # critical path.
                            rs = phb.tile([1, 512], F32, tag="rs",
                                          name="rs")
                            nc.vector.tensor_copy(rs,
                                                  pctxs[h][DV:DV + 1, :])
                            ctu = phb.tile([64, 512], BF16, tag="ctu",
                                           name=f"ctu{h}")
                            nc.vector.tensor_copy(ctu, pctxs[h][0:DV, :])
                            inv = phb.tile([1, 512], F32, tag="inv",
                                           name="inv")
                            nc.vector.reciprocal_approx_fast(
                                out=inv, in_=rs)
                            bc = phb.tile([64, 512], F32, tag="bc",
                                          name="bc")
                            nc.gpsimd.partition_broadcast(out_ap=bc,
                                                          in_ap=inv)
                            nc.vector.tensor_mul(
                                ct[po:po + DV, hp, j * 512:(j + 1) * 512],
                                ctu, bc)
                    if hp == NHP - 1:
                        if j == last_j:
                            em.add_finish(0)
                            em.add_finish(1)
                            em.add_finish(2)
                            em.add_finish(3)
                        else:
                            for lt in range(4 * j, 4 * j + 4):
                                em.add_lt(lt)
                em.drain()

    nc.compile()
    return nc


def _bf16(a):
    import ml_dtypes
    return np.ascontiguousarray(a).astype(ml_dtypes.bfloat16)


def make_in_maps(x, Wq, Wk, Wv, Wo):
    nch = L // 512
    in_maps = []
    for c in range(N_CORES):
        b, g = c // 2, c % 2
        # x^T chunk-major: [c, p, t, lc] = x[b][c*512+lc, t*128+p]
        xdev = x[b].reshape(nch, 512, 8, 128).transpose(0, 3, 2, 1)
        wqg = Wq[:, g * OC:(g + 1) * OC]
        wkg = Wk[:, g * OC:(g + 1) * OC]
        # per-head-pair [hp, p, t, o] = W[t*128+p, hp*128+o]
        wqd = wqg.reshape(8, 128, NHP, 128).transpose(2, 1, 0, 3)
        wkd = wkg.reshape(8, 128, NHP, 128).transpose(2, 1, 0, 3)
        # [p, t, o] = W[t*128+p, o]
        wvd = Wv[:, g * OC:(g + 1) * OC].reshape(8, 128, OC).transpose(1, 0, 2)
        # [p, v, o] = Wo[v*128+p, o]
        wod = Wo[g * OC:(g + 1) * OC, :].reshape(4, 128, D).transpose(1, 0, 2)
        in_maps.append({
            "x": _bf16(xdev),
            "wq": _bf16(wqd),
            "wk": _bf16(wkd),
            "wv": _bf16(wvd),
            "wo": _bf16(wod),
        })
    return in_maps


_NC_CACHE = {}


def _get_nc():
    if "nc" not in _NC_CACHE:
        _NC_CACHE["nc"] = build_nc()
    return _NC_CACHE["nc"]


def _numpy_fallback(x, Wq, Wk, Wv, Wo, bo, mask):
    Bsz, Lq, _ = x.shape
    Q = (x @ Wq).reshape(Bsz, Lq, N_HEAD, DH).transpose(0, 2, 1, 3)
    K = (x @ Wk).reshape(Bsz, Lq, N_HEAD, DH).transpose(0, 2, 1, 3)
    V = (x @ Wv).reshape(Bsz, Lq, N_HEAD, DV).transpose(0, 2, 1, 3)
    s = np.einsum("bhqd,bhkd->bhqk", Q, K) / np.sqrt(np.float32(DH))
    s = np.where(mask, s, -np.inf)
    s = s - s.max(axis=-1, keepdims=True)
    p = np.exp(s)
    p /= p.sum(axis=-1, keepdims=True)
    ctxv = np.einsum("bhqk,bhkv->bhqv", p, V)
    ctxv = ctxv.transpose(0, 2, 1, 3).reshape(Bsz, Lq, N_HEAD * DV)
    return (ctxv @ Wo + bo).astype(np.float32)


def run_on_hw(in_maps, trace=False):
    from concourse.bass_utils import run_bass_kernel_spmd
    nc = _get_nc()
    return run_bass_kernel_spmd(nc, in_maps, list(range(N_CORES)), trace=trace)


def kernel(x, Wq, Wk, Wv, Wo, bo, mask, _trace=False, _results=None):
    x = np.asarray(x, dtype=np.float32)
    Wq = np.asarray(Wq, dtype=np.float32)
    Wk = np.asarray(Wk, dtype=np.float32)
    Wv = np.asarray(Wv, dtype=np.float32)
    Wo = np.asarray(Wo, dtype=np.float32)
    bo = np.asarray(bo, dtype=np.float32)
    mask_np = np.asarray(mask).reshape(mask.shape[-2], mask.shape[-1])

    causal = bool(np.array_equal(
        mask_np, np.tril(np.ones((L, L), dtype=bool))))
    if not causal or x.shape != (B, L, D):
        return _numpy_fallback(np.asarray(x), Wq, Wk, Wv, Wo, bo,
                               np.asarray(mask))

    res = run_on_hw(make_in_maps(x, Wq, Wk, Wv, Wo), trace=_trace)
    if _results is not None:
        _results.append(res)
    out = np.empty((B, L, D), dtype=np.float32)
    for b in range(B):
        out[b] = (np.asarray(res.results[2 * b]["out"], dtype=np.float32)
                  + np.asarray(res.results[2 * b + 1]["out"], dtype=np.float32)
                  + bo)
    return out
